# revision 1
# baseline (speedup 1.0000x reference)
"""DenseCapsule dynamic-routing kernel for 8 Trainium2 NeuronCores.

Strategy (contraction/n sharding, full batch per core):
  - x_hat is never materialized. All routing contractions are expressed
    through the shared weight W so the PE does the heavy lifting:
      s[b,(o,i)]   = sum_f W2[f,(o,i)] * (c  (*) x)[f,b]     (f = (n,j))
      t~[o][f,b]   = sum_i W2[f,(o,i)] * (g*s)[(o,i),b]
      b_inc[o][n,b]= sum_j x[f,b] * t~[o][f,b]               (block-diag PE reduce)
  - Each core owns n in [144k, 144k+144) -> f-rows 1152 = 9 chunks of 128.
    Full batch B=512 rides in the matmul free dim (N=512).
  - s partials are AllReduced across the 8 cores (iters 0,1); the final
    iteration's partial sums + squash happen on the host.
  - squash(s) = g(|s|^2) * s is folded into the t~ matmul moving operand,
    with g computed via Ln/Exp (one ACT table set, no Sqrt set switch).
"""

import sys

sys.path.insert(0, "/opt/trn_rl_repo")

import numpy as np
import ml_dtypes

import concourse.bass as bass  # noqa: F401
import concourse.tile as tile
from concourse import bacc, mybir
from concourse.bass_utils import run_bass_kernel_spmd

B, N_IN, D_IN, N_OUT, D_OUT = 512, 1152, 8, 10, 16
NCORES = 8
NLOC = N_IN // NCORES  # 144
F = NLOC * D_IN        # 1152 f-rows per core, f = 8*n_within + j
NCH = F // 128         # 9 chunks
OI = N_OUT * D_OUT     # 160
BF16 = mybir.dt.bfloat16
F32 = mybir.dt.float32
AF = mybir.ActivationFunctionType
ALU = mybir.AluOpType
bfnp = ml_dtypes.bfloat16

_built = None


def _build():
    nc = bacc.Bacc("TRN2", target_bir_lowering=False, debug=False, num_devices=NCORES)

    xT_d = nc.dram_tensor("xT", [F, B], BF16, kind="ExternalInput")
    w2_d = nc.dram_tensor("w2", [F, OI], BF16, kind="ExternalInput")
    w2t_d = nc.dram_tensor("w2t", [384, F], BF16, kind="ExternalInput")
    w2p_d = nc.dram_tensor("w2p", [F, 320], BF16, kind="ExternalInput")
    bd_d = nc.dram_tensor("bd", [128, 8 * 128], BF16, kind="ExternalInput")
    osel_d = nc.dram_tensor("osel", [384, 16], BF16, kind="ExternalInput")
    out_d = nc.dram_tensor("out", [OI, B], BF16, kind="ExternalOutput")

    with tile.TileContext(nc) as tc, nc.allow_low_precision(
            reason="bf16 softmax/routing logits are within tolerance"):
        _emit(tc, nc, xT_d, w2_d, w2t_d, w2p_d, bd_d, osel_d, out_d)
    nc.compile()
    return nc


def _emit(tc, nc, xT_d, w2_d, w2t_d, w2p_d, bd_d, osel_d, out_d):
    from contextlib import ExitStack

    ctx = ExitStack()
    const = ctx.enter_context(tc.tile_pool(name="const", bufs=1))
    small = ctx.enter_context(tc.tile_pool(name="small", bufs=1))
    cxp = ctx.enter_context(tc.tile_pool(name="cx", bufs=4))
    yp = ctx.enter_context(tc.tile_pool(name="y", bufs=4))
    pp = ctx.enter_context(tc.tile_pool(name="p", bufs=4))
    tsbp = ctx.enter_context(tc.tile_pool(name="tsb", bufs=4))
    psp = ctx.enter_context(tc.tile_pool(name="psp", bufs=8, space="PSUM"))
    dram = ctx.enter_context(tc.tile_pool(name="dram", bufs=1, space="DRAM"))


    # ---- collective warmup (no deps; overlaps the prologue) ----
    wu_in = dram.tile([16, 16], F32, tag="wu_in", name="wu_in")
    wu_out = dram.tile([16, 16], F32, tag="wu_out", name="wu_out")
    nc.gpsimd.collective_compute(
        "AllReduce", ALU.add, replica_groups=[list(range(NCORES))],
        ins=[wu_in.opt()], outs=[wu_out.opt()],
    )

    # ---- load constants ----
    xT = []
    for c in range(NCH):
        t = const.tile([128, B], BF16, tag=f"xT{c}", name=f"xT{c}")
        (nc.sync if c % 2 else nc.scalar).dma_start(t[:], xT_d[128 * c:128 * (c + 1), :])
        xT.append(t)
    w2tp = []
    w2p = []
    oselg = []
    for g in range(3):
        t = const.tile([128, F], BF16, tag=f"w2tp{g}", name=f"w2tp{g}")
        (nc.sync if g % 2 else nc.scalar).dma_start(t[:], w2t_d[128 * g:128 * (g + 1), :])
        w2tp.append(t)
        t2 = const.tile([128, 16], BF16, tag=f"oselg{g}", name=f"oselg{g}")
        nc.sync.dma_start(t2[:], osel_d[128 * g:128 * (g + 1), :])
        oselg.append(t2)
    for c in range(NCH):
        t = const.tile([128, 320], BF16, tag=f"w2p{c}", name=f"w2p{c}")
        (nc.sync if c % 2 else nc.scalar).dma_start(t[:], w2p_d[128 * c:128 * (c + 1), :])
        w2p.append(t)
    bd = const.tile([128, 8 * 128], BF16, tag="bd", name="bd")
    nc.sync.dma_start(bd[:], bd_d[:])

    # ---- persistent per-routing tiles ----
    OB = N_OUT * B  # 5120
    s_red3 = []
    sTg3 = []
    grep3 = []
    sq3 = []
    s_part3 = []
    for g in range(3):
        r = small.tile([128, B], BF16, tag=f"sred3{g}", name=f"sred3{g}")
        nc.gpsimd.memset(r[:], 0.0)
        s_red3.append(r)
        r = small.tile([128, B], BF16, tag=f"sTg3{g}", name=f"sTg3{g}")
        nc.gpsimd.memset(r[:], 0.0)
        sTg3.append(r)
        r = small.tile([128, B], BF16, tag=f"grep3{g}", name=f"grep3{g}")
        nc.gpsimd.memset(r[:], 0.0)
        grep3.append(r)
        r = small.tile([128, B], BF16, tag=f"sq3{g}", name=f"sq3{g}")
        nc.gpsimd.memset(r[:], 0.0)
        sq3.append(r)
        r = small.tile([128, B], BF16, tag=f"spart3{g}", name=f"spart3{g}")
        s_part3.append(r)
    state_a = [small.tile([128, OB], BF16, tag=f"sta{t}", name=f"sta{t}") for t in range(2)]
    state_b = [small.tile([16, OB], BF16, tag=f"stb{t}", name=f"stb{t}") for t in range(2)]
    e_a = small.tile([128, OB], BF16, tag="e_a", name="e_a")
    e_b = small.tile([16, OB], BF16, tag="e_b", name="e_b")

    ar_in = {t: dram.tile([OI, B], BF16, tag=f"arin{t}", name=f"arin{t}") for t in (0, 1)}
    ar_out = {t: dram.tile([OI, B], BF16, tag=f"arout{t}", name=f"arout{t}") for t in (0, 1)}
    c_dram = dram.tile([NLOC, OB], BF16, tag="cdram", name="cdram")
    g_dram = [dram.tile([16, B], BF16, tag=f"gdram{t}", name=f"gdram{t}") for t in range(2)]

    def sl(o):
        return slice(B * o, B * (o + 1))

    # ====== iteration 0: s0 partial = sum_{f local} W2 * x, then AllReduce ==
    w2l = []
    for c in range(NCH):
        t = const.tile([128, OI], BF16, tag=f"w2l{c}", name=f"w2l{c}")
        (nc.scalar if c % 2 else nc.sync).dma_start(
            t[:], w2_d[128 * c:128 * (c + 1), :])
        w2l.append(t)
    p0a = psp.tile([128, B], F32, tag="ps", name="s0a")
    p0b = psp.tile([32, B], F32, tag="ps", name="s0b")
    for c in range(NCH):
        nc.tensor.matmul(p0a[:], w2l[c][:, 0:128], xT[c][:],
                         start=(c == 0), stop=(c == NCH - 1))
    for c in range(NCH):
        nc.tensor.matmul(p0b[:], w2l[c][:, 128:160], xT[c][:],
                         start=(c == 0), stop=(c == NCH - 1))
    s0sb_a = small.tile([128, B], BF16, tag="s0sba", name="s0sba")
    s0sb_b = small.tile([32, B], BF16, tag="s0sbb", name="s0sbb")
    nc.scalar.copy(s0sb_a[:], p0a[:])
    nc.scalar.copy(s0sb_b[:], p0b[:])
    nc.sync.dma_start(ar_in[0][0:128, :], s0sb_a[:])
    nc.sync.dma_start(ar_in[0][128:160, :], s0sb_b[:])
    nc.gpsimd.collective_compute(
        "AllReduce", ALU.add, replica_groups=[list(range(NCORES))],
        ins=[ar_in[0].opt()], outs=[ar_out[0].opt()],
    )
    for o in range(N_OUT):
        g, u = o // 4, o % 4
        nc.sync.dma_start(s_red3[g][32 * u:32 * u + 16, :],
                          ar_out[0][16 * o:16 * (o + 1), :])

    def g_chain(t, alpha):
        """ps_n2 <- |s|^2 per o; grep3 <- repeated ghat rows; sTg3 <- ghat*s."""
        pn2 = psp.tile([16, B], F32, tag="ps", name="n2")
        for g in range(3):
            nc.vector.tensor_mul(sq3[g][:], s_red3[g][:], s_red3[g][:])
            nc.tensor.matmul(pn2[:], oselg[g][:], sq3[g][:],
                             start=(g == 0), stop=(g == 2))
        a2 = float(alpha * alpha)
        g_ln = small.tile([16, B], F32, tag=f"gln{t}", name=f"gln{t}")
        nc.scalar.activation(g_ln[:], pn2[:], AF.Ln, scale=a2)
        g_rt = small.tile([16, B], F32, tag=f"grt{t}", name=f"grt{t}")
        nc.scalar.activation(g_rt[:], g_ln[:], AF.Exp, scale=0.5)
        # ghat = alpha * sqrt(n2) / (1 + n2); with rt = sqrt(n2)/alpha... fold
        # alpha into the denominator: (1 + a2*n2raw)/alpha = alpha*n2raw + 1/alpha
        g_d = small.tile([16, B], F32, tag=f"gd{t}", name=f"gd{t}")
        nc.vector.tensor_scalar(g_d[:], pn2[:], float(alpha), 1.0 / float(alpha),
                                ALU.mult, ALU.add)
        g_r = small.tile([16, B], F32, tag=f"gr{t}", name=f"gr{t}")
        nc.vector.reciprocal(g_r[:], g_d[:])
        g_hat = small.tile([16, B], BF16, tag=f"ghat{t}", name=f"ghat{t}")
        nc.vector.tensor_mul(g_hat[:], g_rt[:], g_r[:])
        # replicate ghat rows (o on rows) to 16-row blocks via DRAM bounce
        nc.sync.dma_start(g_dram[t][:], g_hat[:])
        for o in range(N_OUT):
            g, u = o // 4, o % 4
            nc.sync.dma_start(
                grep3[g][32 * u:32 * u + 16, :],
                g_dram[t][o:o + 1, :].broadcast_to((16, B)),
            )
        for g in range(3):
            nc.vector.tensor_mul(sTg3[g][:], grep3[g][:], s_red3[g][:])

    def agreement(t):
        """state[t] <- (t? state[t-1] : 0) + ghat (.) sum_j x*t~  (all o)."""
        for g in range(3):
            nu = 4 if g < 2 else 2
            pba = {}
            for u in range(nu):
                pba[u] = psp.tile([128, B], F32, tag="ps", name="ba")
            pbb = {}
            for u in range(nu):
                pbb[u] = psp.tile([16, B], F32, tag="ps", name="bb")
            for c in range(NCH):
                for u0 in range(0, nu, 2):
                    pts = {}
                    for u in (u0, u0 + 1):
                        if u >= nu:
                            continue
                        pt = psp.tile([128, B], F32, tag="ps", name="t")
                        nc.tensor.matmul(
                            pt[:], w2tp[g][32 * u:32 * (u + 1), 128 * c:128 * (c + 1)],
                            sTg3[g][32 * u:32 * (u + 1), :],
                            start=True, stop=True, tile_position=(32 * u, 0))
                        pts[u] = pt
                    for u in pts:
                        tsb = tsbp.tile([128, B], BF16, tag="tsb", name="tsb")
                        if u % 2 == 0:
                            nc.scalar.copy(tsb[:], pts[u][:])
                        else:
                            nc.vector.tensor_copy(tsb[:], pts[u][:])
                        p = pp.tile([128, B], BF16, tag="p", name="p")
                        nc.vector.tensor_mul(p[:], tsb[:], xT[c][:])
                        if c < 8:
                            nc.tensor.matmul(pba[u][:], bd[:, 128 * c:128 * (c + 1)],
                                             p[:], start=(c == 0), stop=(c == 7))
                        else:
                            nc.tensor.matmul(pbb[u][:], bd[:, 0:16], p[:],
                                             start=True, stop=True)
            for u in range(nu):
                o = 4 * g + u
                if t == 0:
                    nc.scalar.copy(state_a[0][:, sl(o)], pba[u][:])
                    nc.scalar.copy(state_b[0][:, sl(o)], pbb[u][:])
                else:
                    nc.vector.scalar_tensor_tensor(
                        state_a[1][:, sl(o)], pba[u][:], 1.0,
                        state_a[0][:, sl(o)], op0=ALU.mult, op1=ALU.add)
                    nc.vector.scalar_tensor_tensor(
                        state_b[1][:, sl(o)], pbb[u][:], 1.0,
                        state_b[0][:, sl(o)], op0=ALU.mult, op1=ALU.add)

    def softmax(t):
        """e_a/e_b <- softmax over o of state[t] (written in place as c)."""
        for o in range(N_OUT):
            nc.scalar.activation(e_a[:, sl(o)], state_a[t][:, sl(o)], AF.Exp)
        nc.scalar.activation(e_b[:], state_b[t][:], AF.Exp)
        z_a = small.tile([128, B], BF16, tag=f"za{t}", name=f"za{t}")
        z_b = small.tile([16, B], BF16, tag=f"zb{t}", name=f"zb{t}")
        nc.vector.tensor_copy(z_a[:], e_a[:, sl(0)])
        nc.vector.tensor_copy(z_b[:], e_b[:, sl(0)])
        for o in range(1, N_OUT):
            nc.vector.tensor_add(z_a[:], z_a[:], e_a[:, sl(o)])
            nc.vector.tensor_add(z_b[:], z_b[:], e_b[:, sl(o)])
        zi_a = small.tile([128, B], BF16, tag=f"zia{t}", name=f"zia{t}")
        zi_b = small.tile([16, B], BF16, tag=f"zib{t}", name=f"zib{t}")
        nc.vector.reciprocal(zi_a[:], z_a[:])
        nc.vector.reciprocal(zi_b[:], z_b[:])
        ea3 = e_a[:].rearrange("p (o b) -> p o b", o=N_OUT)
        eb3 = e_b[:].rearrange("p (o b) -> p o b", o=N_OUT)
        nc.vector.tensor_mul(
            ea3, ea3, zi_a[:].unsqueeze(1).broadcast_to((128, N_OUT, B)))
        nc.vector.tensor_mul(
            eb3, eb3, zi_b[:].unsqueeze(1).broadcast_to((16, N_OUT, B)))
        nc.sync.dma_start(c_dram[0:128, :], e_a[:])
        nc.scalar.dma_start(c_dram[128:NLOC, :], e_b[:])

    def y_s_phase(itn):
        """s_part3[g] rows 32u:+16 <- sum_f W2[f,(o,:)] * (c (.) x)[f,:], o=4g+u."""
        for (w0, nw) in ((0, 8), (8, 2)):
            ngrp = nw // 4 if nw >= 4 else 1
            psos = [psp.tile([128, B], F32, tag="ps", name="so") for _ in range(max(ngrp, 1))]
            for c in range(NCH):
                cx = cxp.tile([128, nw * B], BF16, tag="cx", name="cx")
                dma_eng = nc.sync if c % 2 == 0 else nc.scalar
                dma_eng.dma_start(
                    cx[:],
                    c_dram[16 * c:16 * (c + 1),
                           B * w0:B * (w0 + nw)].unsqueeze(1).broadcast_to(
                        (16, 8, nw * B)),
                )
                for uu in range(nw):
                    o = w0 + uu
                    u = uu % 4
                    y = yp.tile([128, B], BF16, tag="y", name="y")
                    nc.vector.tensor_mul(y[:], xT[c][:], cx[:, B * uu:B * (uu + 1)])
                    nc.tensor.matmul(psos[uu // 4][32 * u:32 * (u + 1), :],
                                     w2p[c][:, 32 * o:32 * (o + 1)], y[:],
                                     start=(c == 0), stop=(c == NCH - 1),
                                     tile_position=(0, 32 * u))
            for gg in range(max(ngrp, 1)):
                nc.scalar.copy(s_part3[w0 // 4 + gg][:], psos[gg][:])

    # =====================  routing  =====================================
    g_chain(0, 0.1)
    agreement(0)
    softmax(0)
    y_s_phase(1)

    # ---- AllReduce s1 ----
    for o in range(N_OUT):
        g, u = o // 4, o % 4
        nc.sync.dma_start(ar_in[1][16 * o:16 * (o + 1), :],
                          s_part3[g][32 * u:32 * u + 16, :])
    nc.gpsimd.collective_compute(
        "AllReduce", ALU.add, replica_groups=[list(range(NCORES))],
        ins=[ar_in[1].opt()], outs=[ar_out[1].opt()],
    )
    for o in range(N_OUT):
        g, u = o // 4, o % 4
        nc.sync.dma_start(s_red3[g][32 * u:32 * u + 16, :],
                          ar_out[1][16 * o:16 * (o + 1), :])

    g_chain(1, 1.0)
    agreement(1)
    softmax(1)
    y_s_phase(2)

    # ---- write s2 partials ----
    for o in range(N_OUT):
        g, u = o // 4, o % 4
        nc.sync.dma_start(out_d[16 * o:16 * (o + 1), :],
                          s_part3[g][32 * u:32 * u + 16, :])

    ctx.close()


def _prep_inputs(x, weight):
    """Host-side layout prep. Returns per-core input maps."""
    x = np.asarray(x, dtype=np.float32)
    weight = np.asarray(weight, dtype=np.float32)
    bd_all = np.zeros((128, 8 * 128), dtype=bfnp)
    for cp in range(8):
        for p in range(128):
            bd_all[p, 128 * cp + 16 * cp + p // 8] = 1.0
    # oselg: [3][128, 16]; row p = 32u + i (i<16 live), col m = o = 4g+u
    oselg = np.zeros((3, 128, 16), dtype=bfnp)
    for g in range(3):
        for u in range(4 if g < 2 else 2):
            oselg[g, 32 * u:32 * u + 16, 4 * g + u] = 1.0
    oselg = oselg.reshape(384, 16)
    in_maps = []
    for k in range(NCORES):
        n0, n1 = NLOC * k, NLOC * (k + 1)
        xs = x[:, n0:n1, :]                      # [B, 144, 8]
        xT = np.ascontiguousarray(
            xs.transpose(1, 2, 0).reshape(F, B)).astype(bfnp)
        Wk = weight[:, n0:n1, :, :]              # [10, 144, 16, 8]
        w2 = np.ascontiguousarray(
            Wk.transpose(1, 3, 0, 2).reshape(F, OI)).astype(bfnp)
        w2t = np.ascontiguousarray(w2.T)          # [160, F]
        # w2tp: [3][128, F], rows 32u+0:16 = w2t rows of o=4g+u, rest zero
        w2tp = np.zeros((3, 128, F), dtype=bfnp)
        for g in range(3):
            for u in range(4 if g < 2 else 2):
                o = 4 * g + u
                w2tp[g, 32 * u:32 * u + 16, :] = w2t[16 * o:16 * (o + 1), :]
        w2tp = w2tp.reshape(384, F)
        # w2p: [F, 320], cols 32o+i (i<16) = w2 col 16o+i, rest zero
        w2p = np.zeros((F, 320), dtype=bfnp)
        for o in range(N_OUT):
            w2p[:, 32 * o:32 * o + 16] = w2[:, 16 * o:16 * (o + 1)]
        in_maps.append({
            "xT": xT, "w2": w2.astype(bfnp), "w2t": w2tp,
            "w2p": w2p, "bd": bd_all, "osel": oselg,
        })
    return in_maps


def _squash_np(s):
    norm = np.linalg.norm(s, axis=-1, keepdims=True)
    return (norm ** 2 / (1.0 + norm ** 2) / (norm + 1e-8)) * s


def run_spmd(x, weight, trace=False, tmpdir=None):
    global _built
    if _built is None:
        _built = _build()
    nc = _built
    in_maps = _prep_inputs(x, weight)
    res = run_bass_kernel_spmd(
        nc, in_maps, list(range(NCORES)), trace=trace, tmpdir=tmpdir)
    s2 = np.zeros((OI, B), dtype=np.float32)
    for k in range(NCORES):
        s2 += res.results[k]["out"].astype(np.float32)
    s2 = s2.reshape(N_OUT, D_OUT, B).transpose(2, 0, 1)  # [B, 10, 16]
    out = _squash_np(s2).astype(np.float32)
    return out, res


def kernel(x, weight):
    out, _ = run_spmd(x, weight)
    return out



# revision 8
# speedup vs baseline: 1.1581x; 1.1581x over previous
"""DenseCapsule dynamic-routing kernel for 8 Trainium2 NeuronCores.

Strategy (contraction/n sharding, full batch per core):
  - x_hat is never materialized. All routing contractions go through the
    shared weight W on the PE:
      s[(o,i),b]   = sum_f W2[f,(o,i)] * (c (*) x)[f,b]      (f = (n,j))
      t~[o][f,b]   = sum_i W2[f,(o,i)] * (g*s)[(o,i),b]      (fp8 DoubleRow)
      b_inc[o][n,b]= sum_j x[f,b] * t~[o][f,b]               (block-diag PE)
  - Each core owns n in [144k, 144k+144); full batch B=512 rides in the
    matmul free dim.
  - s partials are AllReduced (iters 0,1); final squash on host.
  - routing logits are never materialized: c2 ~ c1 (*) exp(b_inc1), with
    exp read directly from PSUM on the ACT engine.
  - y = c (*) x runs in n-major layout so c broadcasts via a stride-0 AP
    (no DMA partition-replication); only the 16-row n-tail uses the
    DRAM-broadcast path.
"""

import sys

sys.path.insert(0, "/opt/trn_rl_repo")

import numpy as np
import ml_dtypes

import concourse.bass as bass  # noqa: F401
import concourse.tile as tile
from concourse import bacc, mybir
from concourse.bass_utils import run_bass_kernel_spmd

B, N_IN, D_IN, N_OUT, D_OUT = 512, 1152, 8, 10, 16
NCORES = 8
NLOC = N_IN // NCORES  # 144
F = NLOC * D_IN        # 1152 f-rows per core, f = 8*n_within + j
NCH = F // 128         # 9 chunks
OI = N_OUT * D_OUT     # 160
BF16 = mybir.dt.bfloat16
FP8 = mybir.dt.float8e4
F32 = mybir.dt.float32
AF = mybir.ActivationFunctionType
ALU = mybir.AluOpType
PM = mybir.MatmulPerfMode
bfnp = ml_dtypes.bfloat16
f8np = ml_dtypes.float8_e4m3fn

WT_SCALE = 64.0   # w2tp_dr stored as fp8 * WT_SCALE
GS_SCALE = 16.0   # ghat folded scale so (g*s) fp8 is well-resolved
UNSCALE = 1.0 / (WT_SCALE * GS_SCALE)  # applied in exp(b_inc)

GROUPS = ((0, 4), (4, 4), (8, 2))  # (o0, nu) per group

_built = None


def _build():
    nc = bacc.Bacc("TRN2", target_bir_lowering=False, debug=False, num_devices=NCORES)

    xT_d = nc.dram_tensor("xT", [F, B], BF16, kind="ExternalInput")
    x2_d = nc.dram_tensor("x2", [128, 8 * B], BF16, kind="ExternalInput")
    w2_d = nc.dram_tensor("w2", [F, OI], BF16, kind="ExternalInput")
    wtp_d = nc.dram_tensor("wtp", [384, 2 * F], FP8, kind="ExternalInput")
    w2j_d = nc.dram_tensor("w2j", [8 * 128, 320], BF16, kind="ExternalInput")
    w2pt_d = nc.dram_tensor("w2pt", [128, 320], BF16, kind="ExternalInput")
    bd_d = nc.dram_tensor("bd", [128, 8 * 128], BF16, kind="ExternalInput")
    or2_d = nc.dram_tensor("or2", [128, 128], BF16, kind="ExternalInput")
    out_d = nc.dram_tensor("out", [OI, B], BF16, kind="ExternalOutput")

    with tile.TileContext(nc) as tc, nc.allow_low_precision(
            reason="bf16 routing logits / fp8 agreement path within tolerance"):
        _emit(tc, nc, xT_d, x2_d, w2_d, wtp_d, w2j_d, w2pt_d, bd_d, or2_d, out_d)
    nc.compile()
    return nc


def _emit(tc, nc, xT_d, x2_d, w2_d, wtp_d, w2j_d, w2pt_d, bd_d, or2_d, out_d):
    from contextlib import ExitStack

    ctx = ExitStack()
    const = ctx.enter_context(tc.tile_pool(name="const", bufs=1))
    small = ctx.enter_context(tc.tile_pool(name="small", bufs=1))
    pairp = ctx.enter_context(tc.tile_pool(name="pair", bufs=4))
    yp = ctx.enter_context(tc.tile_pool(name="y", bufs=3))
    # PSUM tags: "pt" (t~ pairs + psos) 2x4KB, "pb" (pba/pn2/p0a) 4KB,
    # "pbt" (pbb/p0b) 4KB -> 16KB total
    psp = ctx.enter_context(tc.tile_pool(name="psp", bufs=1, space="PSUM"))
    dram = ctx.enter_context(tc.tile_pool(name="dram", bufs=1, space="DRAM"))

    # ---- load constants (priority order: s0 path first) ----
    xT = []
    for c in range(NCH):
        t = const.tile([128, B], BF16, tag=f"xT{c}", name=f"xT{c}")
        nc.sync.dma_start(t[:], xT_d[128 * c:128 * (c + 1), :])
        xT.append(t)
    w2l = []
    for c in range(NCH):
        t = const.tile([128, OI], BF16, tag=f"w2l{c}", name=f"w2l{c}")
        nc.scalar.dma_start(t[:], w2_d[128 * c:128 * (c + 1), :])
        w2l.append(t)
    wtp = []
    for g in range(3):
        t = const.tile([128, 2 * F], FP8, tag=f"wtp{g}", name=f"wtp{g}")
        (nc.sync if g % 2 else nc.scalar).dma_start(
            t[:], wtp_d[128 * g:128 * (g + 1), :])
        wtp.append(t)
    or2 = const.tile([128, 128], BF16, tag="or2", name="or2")
    nc.scalar.dma_start(or2[:], or2_d[:])
    bd = const.tile([128, 8 * 128], BF16, tag="bd", name="bd")
    nc.sync.dma_start(bd[:], bd_d[:])
    # y_s-phase constants (needed latest) on the software DGE
    x2 = const.tile([128, 8 * B], BF16, tag="x2", name="x2")
    nc.gpsimd.dma_start(x2[:], x2_d[:])
    w2j = []
    for j in range(8):
        t = const.tile([128, 320], BF16, tag=f"w2j{j}", name=f"w2j{j}")
        nc.gpsimd.dma_start(t[:], w2j_d[128 * j:128 * (j + 1), :])
        w2j.append(t)
    w2pt = const.tile([128, 320], BF16, tag="w2pt", name="w2pt")
    nc.gpsimd.dma_start(w2pt[:], w2pt_d[:])

    # ---- persistent tiles ----
    OB = N_OUT * B  # 5120
    s_red = []   # [g] [128, (t,b)] doublerow layout: row 32u+k = (o=o0+u, i=2k+t)
    sTg = []     # [g] fp8 (g*s) in the same layout
    ghat = []    # [g] [128, 512] replicated squash gain
    sq = []
    for g in range(3):
        r = small.tile([128, 2 * B], BF16, tag=f"sred{g}", name=f"sred{g}")
        nc.gpsimd.memset(r[:], 0.0)
        s_red.append(r)
        sTg.append(small.tile([128, 2 * B], FP8, tag=f"sTg{g}", name=f"sTg{g}"))
        ghat.append(small.tile([128, B], F32, tag=f"ghat{g}", name=f"ghat{g}"))
        sq.append(small.tile([128, 2 * B], BF16, tag=f"sq{g}", name=f"sq{g}"))
    e_a = small.tile([128, OB], BF16, tag="e_a", name="e_a")       # iter-1 c
    e_b = small.tile([16, OB], BF16, tag="e_b", name="e_b")
    f_a = small.tile([128, OB], BF16, tag="f_a", name="f_a")       # iter-2 c
    f_b = small.tile([16, OB], BF16, tag="f_b", name="f_b")
    za5 = small.tile([128, 5 * B], BF16, tag="za5", name="za5")
    zb5 = small.tile([16, 5 * B], BF16, tag="zb5", name="zb5")
    za2 = small.tile([128, 2 * B], BF16, tag="za2", name="za2")
    zb2 = small.tile([16, 2 * B], BF16, tag="zb2", name="zb2")
    s_part3 = [small.tile([128, B], BF16, tag=f"spart{g}", name=f"spart{g}")
               for g in range(3)]
    ln_bias = small.tile([128, 1], F32, tag="lnb", name="ln_bias")
    nc.gpsimd.memset(ln_bias[:], 1e-20)

    ar_in = {t: dram.tile([OI, B], BF16, tag=f"arin{t}", name=f"arin{t}") for t in (0, 1)}
    ar_out = {t: dram.tile([OI, B], BF16, tag=f"arout{t}", name=f"arout{t}") for t in (0, 1)}
    eb_dram = [dram.tile([16, OB], BF16, tag=f"ebd{t}", name=f"ebd{t}") for t in range(2)]

    def sl(o):
        return slice(B * o, B * (o + 1))

    # ====== iteration 0: s0 partial = sum_{f local} W2 * x, then AllReduce ==
    p0a = psp.tile([128, B], F32, tag="pb", bufs=1, name="s0a")
    p0b = psp.tile([32, B], F32, tag="pbt", bufs=1, name="s0b")
    for c in range(NCH):
        nc.tensor.matmul(p0a[:], w2l[c][:, 0:128], xT[c][:],
                         start=(c == 0), stop=(c == NCH - 1))
    for c in range(NCH):
        nc.tensor.matmul(p0b[:], w2l[c][:, 128:160], xT[c][:],
                         start=(c == 0), stop=(c == NCH - 1))
    s0sb_a = small.tile([128, B], BF16, tag="s0sba", name="s0sba")
    s0sb_b = small.tile([32, B], BF16, tag="s0sbb", name="s0sbb")
    nc.scalar.copy(s0sb_a[:], p0a[:])
    nc.scalar.copy(s0sb_b[:], p0b[:])
    nc.sync.dma_start(ar_in[0][0:128, :], s0sb_a[:])
    nc.sync.dma_start(ar_in[0][128:160, :], s0sb_b[:])

    def allreduce(t):
        nc.gpsimd.collective_compute(
            "AllReduce", ALU.add, replica_groups=[list(range(NCORES))],
            ins=[ar_in[t].opt()], outs=[ar_out[t].opt()],
        )
        # scatter into doublerow layout: row 16o+i -> part 32u+k, free (t2,b)
        for g, (o0, nu) in enumerate(GROUPS):
            for u in range(nu):
                o = o0 + u
                src = ar_out[t][16 * o:16 * (o + 1), :].rearrange(
                    "(k t2) b -> k t2 b", k=8)
                dst = s_red[g][32 * u:32 * u + 8, :].rearrange(
                    "k (t2 b) -> k t2 b", t2=2)
                nc.sync.dma_start(dst, src)

    allreduce(0)

    def g_chain(it, alpha):
        """ghat[g] <- replicated GS_SCALE*alpha^2*sqrt(n2)/(1+alpha^2*n2);
        sTg[g] <- fp8 ghat*s."""
        for g in range(3):
            nc.vector.tensor_mul(sq[g][:], s_red[g][:], s_red[g][:])
            pn2 = psp.tile([128, B], F32, tag="pb", bufs=1, name=f"n2_{it}_{g}")
            nc.tensor.matmul(pn2[:], or2[:], sq[g][:, 0:B], start=True, stop=False)
            nc.tensor.matmul(pn2[:], or2[:], sq[g][:, B:2 * B], start=False, stop=True)
            a2 = float(alpha * alpha)
            g_ln = small.tile([128, B], F32, tag=f"gln{g}", name=f"gln{it}{g}")
            nc.scalar.activation(g_ln[:], pn2[:], AF.Ln, scale=a2, bias=ln_bias[:])
            g_rt = small.tile([128, B], F32, tag=f"grt{g}", name=f"grt{it}{g}")
            nc.scalar.activation(g_rt[:], g_ln[:], AF.Exp, scale=0.5)
            g_d = small.tile([128, B], F32, tag=f"gd{g}", name=f"gd{it}{g}")
            nc.vector.tensor_scalar(g_d[:], pn2[:], float(alpha / GS_SCALE),
                                    float(1.0 / (alpha * GS_SCALE)),
                                    ALU.mult, ALU.add)
            g_r = small.tile([128, B], F32, tag=f"gr{g}", name=f"gr{it}{g}")
            nc.vector.reciprocal(g_r[:], g_d[:])
            nc.vector.tensor_mul(ghat[g][:], g_rt[:], g_r[:])
            nc.vector.tensor_mul(
                sTg[g][:].rearrange("p (t2 b) -> p t2 b", t2=2),
                ghat[g][:].unsqueeze(1).broadcast_to((128, 2, B)),
                s_red[g][:].rearrange("p (t2 b) -> p t2 b", t2=2))

    def agreement(it, e_main, e_tail):
        """e_main/e_tail <- exp(UNSCALE * b_inc) (t=0) via ACT from PSUM."""
        for g, (o0, nu) in enumerate(GROUPS):
            for pi in range(nu // 2):
                u0 = 2 * pi
                pba = psp.tile([128, 2 * B], F32, tag="pb", bufs=1, name=f"ba{it}{g}{pi}")
                pbb = psp.tile([16, 2 * B], F32, tag="pbt", bufs=1, name=f"bb{it}{g}{pi}")
                for c in range(NCH):
                    ptp = psp.tile([128, 2 * B], F32, tag="pt", bufs=2, name="pt")
                    for du in range(2):
                        u = u0 + du
                        nc.tensor.matmul(
                            ptp[:, B * du:B * (du + 1)],
                            wtp[g][32 * u:32 * u + 8, :].rearrange(
                                "k (t2 f) -> k t2 f", t2=2)[:, :, 128 * c:128 * (c + 1)],
                            sTg[g][32 * u:32 * u + 8, :].rearrange(
                                "k (t2 b) -> k t2 b", t2=2),
                            start=True, stop=True, perf_mode=PM.DoubleRow,
                            tile_position=(32 * u, 0))
                    # drain: p = t~ (*) x, alternating direct / cast+mul
                    pp = pairp.tile([128, 2 * B], BF16, tag="pp", name="pp")
                    if c % 2 == 0:
                        nc.vector.tensor_mul(
                            pp[:].rearrange("p (u b) -> p u b", u=2),
                            ptp[:].rearrange("p (u b) -> p u b", u=2),
                            xT[c][:].unsqueeze(1).broadcast_to((128, 2, B)))
                    else:
                        tsb = pairp.tile([128, 2 * B], BF16, tag="tsb", name="tsb")
                        nc.scalar.copy(tsb[:], ptp[:])
                        nc.vector.tensor_mul(
                            pp[:].rearrange("p (u b) -> p u b", u=2),
                            tsb[:].rearrange("p (u b) -> p u b", u=2),
                            xT[c][:].unsqueeze(1).broadcast_to((128, 2, B)))
                    for du in range(2):
                        if c < 8:
                            nc.tensor.matmul(pba[:, B * du:B * (du + 1)],
                                             bd[:, 128 * c:128 * (c + 1)],
                                             pp[:, B * du:B * (du + 1)],
                                             start=(c == 0), stop=(c == 7))
                        else:
                            nc.tensor.matmul(pbb[:, B * du:B * (du + 1)],
                                             bd[:, 0:16], pp[:, B * du:B * (du + 1)],
                                             start=True, stop=True)
                o = o0 + u0
                nc.scalar.activation(e_main[:, B * o:B * (o + 2)], pba[:],
                                     AF.Exp, scale=UNSCALE)
                nc.scalar.activation(e_tail[:, B * o:B * (o + 2)], pbb[:],
                                     AF.Exp, scale=UNSCALE)

    def softmax_norm(e_main, e_tail):
        """normalize over o in place: e <- e / sum_o e."""
        for (e, z5, z2, P) in ((e_main, za5, za2, 128), (e_tail, zb5, zb2, 16)):
            nc.vector.tensor_add(z5[:], e[:, 0:5 * B], e[:, 5 * B:10 * B])
            nc.vector.tensor_add(z2[:], z5[:, 0:2 * B], z5[:, 2 * B:4 * B])
            z = small.tile([P, B], BF16, tag=f"z{P}", name=f"z{P}")
            nc.vector.tensor_add(z[:], z2[:, 0:B], z2[:, B:2 * B])
            nc.vector.tensor_add(z[:], z[:], z5[:, 4 * B:5 * B])
            zi = small.tile([P, B], BF16, tag=f"zi{P}", name=f"zi{P}")
            nc.vector.reciprocal(zi[:], z[:])
            e3 = e[:].rearrange("p (o b) -> p o b", o=N_OUT)
            nc.vector.tensor_mul(
                e3, e3, zi[:].unsqueeze(1).broadcast_to((P, N_OUT, B)))

    def y_s_phase(it, e_main, e_tail):
        """s_part3[g] rows 32u:+16 <- sum_f W2[f,(o,:)]*(c (*) x)[f,:], o=o0+u."""
        nc.sync.dma_start(eb_dram[it][:], e_tail[:])
        psos = []
        for g, (o0, nu) in enumerate(GROUPS):
            ps = psp.tile([128, B], F32, tag="pt", bufs=2, name=f"so{it}{g}")
            psos.append(ps)
            for u in range(nu):
                o = o0 + u
                y2 = yp.tile([128, 8 * B], BF16, tag="y2", name="y2")
                nc.vector.tensor_mul(
                    y2[:].rearrange("p (j b) -> p j b", j=8),
                    x2[:].rearrange("p (j b) -> p j b", j=8),
                    e_main[:, sl(o)].unsqueeze(1).broadcast_to((128, 8, B)))
                for j in range(8):
                    nc.tensor.matmul(ps[32 * u:32 * (u + 1), :],
                                     w2j[j][:, 32 * o:32 * (o + 1)],
                                     y2[:, B * j:B * (j + 1)],
                                     start=(j == 0), stop=False,
                                     tile_position=(0, 32 * u))
                cxt = yp.tile([128, B], BF16, tag="cxt", name="cxt")
                nc.sync.dma_start(
                    cxt[:],
                    eb_dram[it][:, sl(o)].unsqueeze(1).broadcast_to((16, 8, B)))
                yt = yp.tile([128, B], BF16, tag="yt", name="yt")
                nc.vector.tensor_mul(yt[:], xT[8][:], cxt[:])
                nc.tensor.matmul(ps[32 * u:32 * (u + 1), :],
                                 w2pt[:, 32 * o:32 * (o + 1)], yt[:],
                                 start=False, stop=True,
                                 tile_position=(0, 32 * u))
            nc.scalar.copy(s_part3[g][:], psos[g][:])

    # =====================  routing  =====================================
    g_chain(0, 0.1)
    agreement(0, e_a, e_b)
    softmax_norm(e_a, e_b)
    y_s_phase(0, e_a, e_b)

    # ---- AllReduce s1 ----
    for g, (o0, nu) in enumerate(GROUPS):
        for u in range(nu):
            o = o0 + u
            nc.sync.dma_start(ar_in[1][16 * o:16 * (o + 1), :],
                              s_part3[g][32 * u:32 * u + 16, :])
    allreduce(1)

    g_chain(1, 1.0)
    agreement(1, f_a, f_b)
    # c2 ~ c1 (*) exp(b_inc1), renormalized
    nc.vector.tensor_mul(f_a[:], f_a[:], e_a[:])
    nc.vector.tensor_mul(f_b[:], f_b[:], e_b[:])
    softmax_norm(f_a, f_b)
    y_s_phase(1, f_a, f_b)

    # ---- write s2 partials ----
    for g, (o0, nu) in enumerate(GROUPS):
        for u in range(nu):
            o = o0 + u
            nc.sync.dma_start(out_d[16 * o:16 * (o + 1), :],
                              s_part3[g][32 * u:32 * u + 16, :])

    ctx.close()


def _prep_inputs(x, weight):
    """Host-side layout prep. Returns per-core input maps."""
    x = np.asarray(x, dtype=np.float32)
    weight = np.asarray(weight, dtype=np.float32)
    # bd: block-diag j-reduce, chunk cp of 8 maps f-row p -> n-row 16cp + p//8
    bd_all = np.zeros((128, 8 * 128), dtype=bfnp)
    for cp in range(8):
        for p in range(128):
            bd_all[p, 128 * cp + 16 * cp + p // 8] = 1.0
    # or2: one-hot row-replicate |s|^2 within each 32-block
    or2 = np.zeros((128, 128), dtype=bfnp)
    for u in range(4):
        or2[32 * u:32 * u + 16, 32 * u:32 * u + 16] = 1.0
    in_maps = []
    for k in range(NCORES):
        n0, n1 = NLOC * k, NLOC * (k + 1)
        xs = x[:, n0:n1, :]                      # [B, 144, 8]
        xT = np.ascontiguousarray(
            xs.transpose(1, 2, 0).reshape(F, B)).astype(bfnp)
        # x2: n-major [n, j, b] for n < 128
        x2 = np.ascontiguousarray(
            xs[:, :128, :].transpose(1, 2, 0).reshape(128, 8 * B)).astype(bfnp)
        Wk = weight[:, n0:n1, :, :]              # [10, 144, 16, 8]
        w2 = np.ascontiguousarray(
            Wk.transpose(1, 3, 0, 2).reshape(F, OI)).astype(bfnp)
        # wtp: doublerow fp8 stationary [3][128, 2, F]:
        #   row 32u+k, subtile t2, col f = WT_SCALE * W2[f, 16*(o0+u) + 2k+t2]
        w2t = w2.astype(np.float32).T            # [160, F]
        wtp = np.zeros((3, 128, 2, F), dtype=f8np)
        for g, (o0, nu) in enumerate(GROUPS):
            for u in range(nu):
                o = o0 + u
                blk = w2t[16 * o:16 * (o + 1), :]          # [16, F] (i, f)
                wtp[g, 32 * u:32 * u + 8, 0, :] = (WT_SCALE * blk[0::2]).astype(f8np)
                wtp[g, 32 * u:32 * u + 8, 1, :] = (WT_SCALE * blk[1::2]).astype(f8np)
        wtp = wtp.reshape(384, 2 * F)
        # w2j: [8][128 n, 320]: col 32o+i = W[o, n, i, j]
        w2j = np.zeros((8, 128, 320), dtype=bfnp)
        for o in range(N_OUT):
            # Wk[o,n,i,j] with n<128
            # Wk[o, n, i, j] -> w2j[j, n, i]
            w2j[:, :, 32 * o:32 * o + 16] = Wk[o, :128].transpose(2, 0, 1)
        w2j = w2j.reshape(8 * 128, 320)
        # w2pt: tail chunk (f rows 1024:1152 = n 128:144), col 32o+i
        w2pt = np.zeros((128, 320), dtype=bfnp)
        for o in range(N_OUT):
            w2pt[:, 32 * o:32 * o + 16] = w2[1024:1152, 16 * o:16 * (o + 1)]
        in_maps.append({
            "xT": xT, "x2": x2, "w2": w2, "wtp": wtp.astype(f8np),
            "w2j": w2j, "w2pt": w2pt, "bd": bd_all, "or2": or2,
        })
    return in_maps


def _squash_np(s):
    norm = np.linalg.norm(s, axis=-1, keepdims=True)
    return (norm ** 2 / (1.0 + norm ** 2) / (norm + 1e-8)) * s


def run_spmd(x, weight, trace=False, tmpdir=None):
    global _built
    if _built is None:
        _built = _build()
    nc = _built
    in_maps = _prep_inputs(x, weight)
    res = run_bass_kernel_spmd(
        nc, in_maps, list(range(NCORES)), trace=trace, tmpdir=tmpdir)
    s2 = np.zeros((OI, B), dtype=np.float32)
    for k in range(NCORES):
        s2 += res.results[k]["out"].astype(np.float32)
    s2 = s2.reshape(N_OUT, D_OUT, B).transpose(2, 0, 1)  # [B, 10, 16]
    out = _squash_np(s2).astype(np.float32)
    return out, res


def kernel(x, weight):
    out, _ = run_spmd(x, weight)
    return out


# revision 11
# speedup vs baseline: 1.2249x; 1.0577x over previous
"""DenseCapsule dynamic-routing kernel for 8 Trainium2 NeuronCores.

Strategy (contraction/n sharding, full batch per core):
  - x_hat is never materialized. All routing contractions go through the
    shared weight W on the PE:
      s[(o,i),b]   = sum_f W2[f,(o,i)] * (c (*) x)[f,b]      (f = (n,j))
      t~[o][f,b]   = sum_i W2[f,(o,i)] * (g*s)[(o,i),b]      (fp8 DoubleRow)
      b_inc[o][n,b]= sum_j x[f,b] * t~[o][f,b]               (block-diag PE)
  - Each core owns n in [144k, 144k+144); full batch B=512 rides in the
    matmul free dim.
  - s partials are AllReduced (iters 0,1); final squash on host.
  - routing logits are never materialized: c2 ~ c1 (*) exp(b_inc1), with
    exp read directly from PSUM on the ACT engine.
  - y = c (*) x runs in n-major layout so c broadcasts via a stride-0 AP
    (no DMA partition-replication); only the 16-row n-tail uses the
    DRAM-broadcast path.
"""

import sys

sys.path.insert(0, "/opt/trn_rl_repo")

import numpy as np
import ml_dtypes

import concourse.bass as bass  # noqa: F401
import concourse.tile as tile
from concourse import bacc, mybir
from concourse.bass_utils import run_bass_kernel_spmd

B, N_IN, D_IN, N_OUT, D_OUT = 512, 1152, 8, 10, 16
NCORES = 8
NLOC = N_IN // NCORES  # 144
F = NLOC * D_IN        # 1152 f-rows per core, f = 8*n_within + j
NCH = F // 128         # 9 chunks
OI = N_OUT * D_OUT     # 160
BF16 = mybir.dt.bfloat16
FP8 = mybir.dt.float8e4
F32 = mybir.dt.float32
AF = mybir.ActivationFunctionType
ALU = mybir.AluOpType
PM = mybir.MatmulPerfMode
bfnp = ml_dtypes.bfloat16
f8np = ml_dtypes.float8_e4m3fn

WT_SCALE = 64.0   # w2tp_dr stored as fp8 * WT_SCALE
GS_SCALE = 16.0   # ghat folded scale so (g*s) fp8 is well-resolved
UNSCALE = 1.0 / (WT_SCALE * GS_SCALE)  # applied in exp(b_inc)

GROUPS = ((0, 4), (4, 4), (8, 2))  # (o0, nu) per group

_built = None


def _build():
    nc = bacc.Bacc("TRN2", target_bir_lowering=False, debug=False, num_devices=NCORES)

    xT_d = nc.dram_tensor("xT", [F, B], BF16, kind="ExternalInput")
    x2_d = nc.dram_tensor("x2", [128, 8 * B], BF16, kind="ExternalInput")
    w2_d = nc.dram_tensor("w2", [F, OI], BF16, kind="ExternalInput")
    wtp_d = nc.dram_tensor("wtp", [384, 2 * F], FP8, kind="ExternalInput")
    w2j_d = nc.dram_tensor("w2j", [8 * 128, 320], BF16, kind="ExternalInput")
    w2pt_d = nc.dram_tensor("w2pt", [128, 320], BF16, kind="ExternalInput")
    bd_d = nc.dram_tensor("bd", [128, 8 * 128], BF16, kind="ExternalInput")
    or2_d = nc.dram_tensor("or2", [128, 128], BF16, kind="ExternalInput")
    out_d = nc.dram_tensor("out", [OI, B], BF16, kind="ExternalOutput")

    with tile.TileContext(nc) as tc, nc.allow_low_precision(
            reason="bf16 routing logits / fp8 agreement path within tolerance"):
        _emit(tc, nc, xT_d, x2_d, w2_d, wtp_d, w2j_d, w2pt_d, bd_d,
              or2_d, out_d)
    nc.compile()
    return nc


def _emit(tc, nc, xT_d, x2_d, w2_d, wtp_d, w2j_d, w2pt_d, bd_d,
          or2_d, out_d):
    from contextlib import ExitStack

    ctx = ExitStack()
    const = ctx.enter_context(tc.tile_pool(name="const", bufs=1))
    small = ctx.enter_context(tc.tile_pool(name="small", bufs=1))
    pairp = ctx.enter_context(tc.tile_pool(name="pair", bufs=4))
    yp = ctx.enter_context(tc.tile_pool(name="y", bufs=3))
    # PSUM tags: "pt" (t~ pairs + psos) 2x4KB, "pb" (pba/pn2/p0a) 4KB,
    # "pbt" (pbb/p0b) 4KB -> 16KB total
    psp = ctx.enter_context(tc.tile_pool(name="psp", bufs=1, space="PSUM"))
    dram = ctx.enter_context(tc.tile_pool(name="dram", bufs=1, space="DRAM"))

    # ---- collective warmup: first in the gpsimd queue, absorbs CC setup ----
    wu_in = dram.tile([16, 16], F32, tag="wu_in", name="wu_in")
    wu_out = dram.tile([16, 16], F32, tag="wu_out", name="wu_out")
    nc.gpsimd.collective_compute(
        "AllReduce", ALU.add, replica_groups=[list(range(NCORES))],
        ins=[wu_in.opt()], outs=[wu_out.opt()],
    )

    # ---- load constants (priority order: s0 path first) ----
    xT = []
    for c in range(NCH):
        t = const.tile([128, B], BF16, tag=f"xT{c}", name=f"xT{c}")
        nc.sync.dma_start(t[:], xT_d[128 * c:128 * (c + 1), :])
        xT.append(t)
    w2l = []
    for c in range(NCH):
        t = const.tile([128, OI], BF16, tag=f"w2l{c}", name=f"w2l{c}")
        nc.scalar.dma_start(t[:], w2_d[128 * c:128 * (c + 1), :])
        w2l.append(t)
    wtp = []
    for g in range(3):
        t = const.tile([128, 2 * F], FP8, tag=f"wtp{g}", name=f"wtp{g}")
        (nc.sync if g % 2 else nc.scalar).dma_start(
            t[:], wtp_d[128 * g:128 * (g + 1), :])
        wtp.append(t)
    or2 = const.tile([128, 128], BF16, tag="or2", name="or2")
    nc.scalar.dma_start(or2[:], or2_d[:])
    bd = const.tile([128, 8 * 128], BF16, tag="bd", name="bd")
    nc.sync.dma_start(bd[:], bd_d[:])
    # y_s-phase constants (needed latest) on the software DGE
    x2 = const.tile([128, 8 * B], BF16, tag="x2", name="x2")
    nc.gpsimd.dma_start(x2[:], x2_d[:])
    w2j = []
    for j in range(8):
        t = const.tile([128, 320], BF16, tag=f"w2j{j}", name=f"w2j{j}")
        nc.gpsimd.dma_start(t[:], w2j_d[128 * j:128 * (j + 1), :])
        w2j.append(t)
    w2pt = const.tile([128, 320], BF16, tag="w2pt", name="w2pt")
    nc.gpsimd.dma_start(w2pt[:], w2pt_d[:])

    # ---- persistent tiles ----
    OB = N_OUT * B  # 5120
    s_red = []   # [g] [128, (t,b)] doublerow layout: row 32u+k = (o=o0+u, i=2k+t)
    sTg = []     # [g] fp8 (g*s) in the same layout
    ghat = []    # [g] [128, 512] replicated squash gain
    sq = []
    for g in range(3):
        r = small.tile([128, 2 * B], BF16, tag=f"sred{g}", name=f"sred{g}")
        nc.gpsimd.memset(r[:], 0.0)
        s_red.append(r)
        sTg.append(small.tile([128, 2 * B], FP8, tag=f"sTg{g}", name=f"sTg{g}"))
        ghat.append(small.tile([128, B], F32, tag=f"ghat{g}", name=f"ghat{g}"))
        sq.append(small.tile([128, 2 * B], BF16, tag=f"sq{g}", name=f"sq{g}"))
    e_a = small.tile([128, OB], BF16, tag="e_a", name="e_a")       # iter-1 c
    e_b = small.tile([16, OB], BF16, tag="e_b", name="e_b")
    f_a = small.tile([128, OB], BF16, tag="f_a", name="f_a")       # iter-2 c
    f_b = small.tile([16, OB], BF16, tag="f_b", name="f_b")
    za5 = small.tile([128, 5 * B], BF16, tag="za5", name="za5")
    zb5 = small.tile([16, 5 * B], BF16, tag="zb5", name="zb5")
    za2 = small.tile([128, 2 * B], BF16, tag="za2", name="za2")
    zb2 = small.tile([16, 2 * B], BF16, tag="zb2", name="zb2")
    s_part3 = [small.tile([128, B], BF16, tag=f"spart{g}", name=f"spart{g}")
               for g in range(3)]
    ln_bias = small.tile([128, 1], F32, tag="lnb", name="ln_bias")
    nc.gpsimd.memset(ln_bias[:], 1e-20)
    inva_bias = small.tile([128, 1], F32, tag="invab", name="inva_bias")
    nc.gpsimd.memset(inva_bias[:], 10.0)
    lngs_bias = small.tile([128, 1], F32, tag="lngsb", name="lngs_bias")
    nc.gpsimd.memset(lngs_bias[:], float(np.log(GS_SCALE)))

    ar_in = {t: dram.tile([OI, B], BF16, tag=f"arin{t}", name=f"arin{t}") for t in (0, 1)}
    ar_out = {t: dram.tile([OI, B], BF16, tag=f"arout{t}", name=f"arout{t}") for t in (0, 1)}
    eb_dram = [dram.tile([16, OB], BF16, tag=f"ebd{t}", name=f"ebd{t}") for t in range(2)]

    def sl(o):
        return slice(B * o, B * (o + 1))

    # ====== iteration 0: s0 partial = sum_{f local} W2 * x, then AllReduce ==
    p0a = psp.tile([128, B], F32, tag="pb", bufs=1, name="s0a")
    p0b = psp.tile([32, B], F32, tag="pbt", bufs=1, name="s0b")
    for c in range(NCH):
        nc.tensor.matmul(p0a[:], w2l[c][:, 0:128], xT[c][:],
                         start=(c == 0), stop=(c == NCH - 1))
    for c in range(NCH):
        nc.tensor.matmul(p0b[:], w2l[c][:, 128:160], xT[c][:],
                         start=(c == 0), stop=(c == NCH - 1))
    s0sb_a = small.tile([128, B], BF16, tag="s0sba", name="s0sba")
    s0sb_b = small.tile([32, B], BF16, tag="s0sbb", name="s0sbb")
    nc.scalar.copy(s0sb_a[:], p0a[:])
    nc.scalar.copy(s0sb_b[:], p0b[:])
    nc.sync.dma_start(ar_in[0][0:128, :], s0sb_a[:])
    nc.sync.dma_start(ar_in[0][128:160, :], s0sb_b[:])

    def allreduce(t):
        nc.gpsimd.collective_compute(
            "AllReduce", ALU.add, replica_groups=[list(range(NCORES))],
            ins=[ar_in[t].opt()], outs=[ar_out[t].opt()],
        )
        # scatter into doublerow layout: row 16o+i -> part 32u+k, free (t2,b)
        for g, (o0, nu) in enumerate(GROUPS):
            for u in range(nu):
                o = o0 + u
                src = ar_out[t][16 * o:16 * (o + 1), :].rearrange(
                    "(k t2) b -> k t2 b", k=8)
                dst = s_red[g][32 * u:32 * u + 8, :].rearrange(
                    "k (t2 b) -> k t2 b", t2=2)
                (nc.sync if (g + u) % 2 == 0 else nc.scalar).dma_start(dst, src)

    allreduce(0)

    def g_chain(it, alpha):
        """ghat[g] <- GS * alpha^2 sqrt(n2)/(1+alpha^2 n2) replicated;
        sTg[g] <- fp8 ghat*s.  All transcendentals on ACT (Ln/Exp)."""
        a2 = float(alpha * alpha)
        inva = inva_bias[:] if alpha != 1.0 else 1.0
        pn2s, l1s, l2s, diffs = [], [], [], []
        for g in range(3):
            nc.vector.tensor_mul(sq[g][:], s_red[g][:], s_red[g][:])
            pn2 = psp.tile([128, B], F32, tag="pt", bufs=2, name=f"n2_{it}_{g}")
            nc.tensor.matmul(pn2[:], or2[:], sq[g][:, 0:B], start=True, stop=False)
            nc.tensor.matmul(pn2[:], or2[:], sq[g][:, B:2 * B], start=False, stop=True)
            pn2s.append(pn2)
            l1 = small.tile([128, B], F32, tag=f"gl1{g}", name=f"gl1{it}{g}")
            nc.scalar.activation(l1[:], pn2[:], AF.Ln, scale=a2, bias=ln_bias[:])
            l2 = small.tile([128, B], F32, tag=f"gl2{g}", name=f"gl2{it}{g}")
            nc.scalar.activation(l2[:], pn2[:], AF.Ln, scale=float(alpha), bias=inva)
            l1s.append(l1)
            l2s.append(l2)
        for g in range(3):
            d = small.tile([128, B], F32, tag=f"gdf{g}", name=f"gdf{it}{g}")
            nc.vector.scalar_tensor_tensor(d[:], l1s[g][:], 0.5, l2s[g][:],
                                           op0=ALU.mult, op1=ALU.subtract)
            diffs.append(d)
        for g in range(3):
            nc.scalar.activation(ghat[g][:], diffs[g][:], AF.Exp,
                                 bias=lngs_bias[:])
        for g in range(3):
            nc.vector.tensor_mul(
                sTg[g][:].rearrange("p (t2 b) -> p t2 b", t2=2),
                ghat[g][:].unsqueeze(1).broadcast_to((128, 2, B)),
                s_red[g][:].rearrange("p (t2 b) -> p t2 b", t2=2))

    def agreement(it, e_main, e_tail):
        """e_main/e_tail <- exp(UNSCALE * b_inc) (t=0) via ACT from PSUM."""
        for g, (o0, nu) in enumerate(GROUPS):
            for pi in range(nu // 2):
                u0 = 2 * pi
                pba = psp.tile([128, 2 * B], F32, tag="pb", bufs=1, name=f"ba{it}{g}{pi}")
                pbb = psp.tile([16, 2 * B], F32, tag="pbt", bufs=1, name=f"bb{it}{g}{pi}")
                for c in range(NCH):
                    ptp = psp.tile([128, 2 * B], F32, tag="pt", bufs=2, name="pt")
                    for du in range(2):
                        u = u0 + du
                        nc.tensor.matmul(
                            ptp[:, B * du:B * (du + 1)],
                            wtp[g][32 * u:32 * u + 8, :].rearrange(
                                "k (t2 f) -> k t2 f", t2=2)[:, :, 128 * c:128 * (c + 1)],
                            sTg[g][32 * u:32 * u + 8, :].rearrange(
                                "k (t2 b) -> k t2 b", t2=2),
                            start=True, stop=True, perf_mode=PM.DoubleRow,
                            tile_position=(32 * u, 0))
                    # drain: p = t~ (*) x, alternating direct / cast+mul
                    pp = pairp.tile([128, 2 * B], BF16, tag="pp", name="pp")
                    if c % 2 == 0:
                        nc.vector.tensor_mul(
                            pp[:].rearrange("p (u b) -> p u b", u=2),
                            ptp[:].rearrange("p (u b) -> p u b", u=2),
                            xT[c][:].unsqueeze(1).broadcast_to((128, 2, B)))
                    else:
                        tsb = pairp.tile([128, 2 * B], BF16, tag="tsb", name="tsb")
                        nc.scalar.copy(tsb[:], ptp[:])
                        nc.vector.tensor_mul(
                            pp[:].rearrange("p (u b) -> p u b", u=2),
                            tsb[:].rearrange("p (u b) -> p u b", u=2),
                            xT[c][:].unsqueeze(1).broadcast_to((128, 2, B)))
                    for du in range(2):
                        if c < 8:
                            nc.tensor.matmul(pba[:, B * du:B * (du + 1)],
                                             bd[:, 128 * c:128 * (c + 1)],
                                             pp[:, B * du:B * (du + 1)],
                                             start=(c == 0), stop=(c == 7))
                        else:
                            nc.tensor.matmul(pbb[:, B * du:B * (du + 1)],
                                             bd[:, 0:16], pp[:, B * du:B * (du + 1)],
                                             start=True, stop=True)
                o = o0 + u0
                nc.scalar.activation(e_main[:, B * o:B * (o + 2)], pba[:],
                                     AF.Exp, scale=UNSCALE)
                nc.scalar.activation(e_tail[:, B * o:B * (o + 2)], pbb[:],
                                     AF.Exp, scale=UNSCALE)

    def softmax_norm(e_main, e_tail):
        """normalize over o in place: e <- e / sum_o e."""
        for (e, z5, z2, P) in ((e_main, za5, za2, 128), (e_tail, zb5, zb2, 16)):
            nc.vector.tensor_add(z5[:], e[:, 0:5 * B], e[:, 5 * B:10 * B])
            nc.vector.tensor_add(z2[:], z5[:, 0:2 * B], z5[:, 2 * B:4 * B])
            z = small.tile([P, B], BF16, tag=f"z{P}", name=f"z{P}")
            nc.vector.tensor_add(z[:], z2[:, 0:B], z2[:, B:2 * B])
            nc.vector.tensor_add(z[:], z[:], z5[:, 4 * B:5 * B])
            zl = small.tile([P, B], F32, tag=f"zl{P}", name=f"zl{P}")
            nc.scalar.activation(zl[:], z[:], AF.Ln)
            zi = small.tile([P, B], BF16, tag=f"zi{P}", name=f"zi{P}")
            nc.scalar.activation(zi[:], zl[:], AF.Exp, scale=-1.0)
            e3 = e[:].rearrange("p (o b) -> p o b", o=N_OUT)
            nc.vector.tensor_mul(
                e3, e3, zi[:].unsqueeze(1).broadcast_to((P, N_OUT, B)))

    def y_s_phase(it, e_main, e_tail):
        """s_part3[g] rows 32u:+16 <- sum_f W2[f,(o,:)]*(c (*) x)[f,:], o=o0+u."""
        nc.sync.dma_start(eb_dram[it][:], e_tail[:])
        psos = []
        for g, (o0, nu) in enumerate(GROUPS):
            ps = psp.tile([128, B], F32, tag="pt", bufs=2, name=f"so{it}{g}")
            psos.append(ps)
            for u in range(nu):
                o = o0 + u
                y2 = yp.tile([128, 8 * B], BF16, tag="y2", name="y2")
                nc.vector.tensor_mul(
                    y2[:].rearrange("p (j b) -> p j b", j=8),
                    x2[:].rearrange("p (j b) -> p j b", j=8),
                    e_main[:, sl(o)].unsqueeze(1).broadcast_to((128, 8, B)))
                for j in range(8):
                    nc.tensor.matmul(ps[32 * u:32 * (u + 1), :],
                                     w2j[j][:, 32 * o:32 * (o + 1)],
                                     y2[:, B * j:B * (j + 1)],
                                     start=(j == 0), stop=False,
                                     tile_position=(0, 32 * u))
                cxt = yp.tile([128, B], BF16, tag="cxt", name="cxt")
                nc.sync.dma_start(
                    cxt[:],
                    eb_dram[it][:, sl(o)].unsqueeze(1).broadcast_to((16, 8, B)))
                yt = yp.tile([128, B], BF16, tag="yt", name="yt")
                nc.vector.tensor_mul(yt[:], xT[8][:], cxt[:])
                nc.tensor.matmul(ps[32 * u:32 * (u + 1), :],
                                 w2pt[:, 32 * o:32 * (o + 1)], yt[:],
                                 start=False, stop=True,
                                 tile_position=(0, 32 * u))
            nc.scalar.copy(s_part3[g][:], psos[g][:])
            if it == 0:
                for u in range(nu):
                    o = o0 + u
                    nc.sync.dma_start(ar_in[1][16 * o:16 * (o + 1), :],
                                      s_part3[g][32 * u:32 * u + 16, :])

    # =====================  routing  =====================================
    g_chain(0, 0.1)
    agreement(0, e_a, e_b)
    softmax_norm(e_a, e_b)
    y_s_phase(0, e_a, e_b)

    # ---- AllReduce s1 (staging DMAs issued per-group inside y_s_phase) ----
    allreduce(1)

    g_chain(1, 1.0)
    agreement(1, f_a, f_b)
    # c2 ~ c1 (*) exp(b_inc1), renormalized
    nc.vector.tensor_mul(f_a[:], f_a[:], e_a[:])
    nc.vector.tensor_mul(f_b[:], f_b[:], e_b[:])
    softmax_norm(f_a, f_b)
    y_s_phase(1, f_a, f_b)

    # ---- write s2 partials ----
    for g, (o0, nu) in enumerate(GROUPS):
        for u in range(nu):
            o = o0 + u
            nc.sync.dma_start(out_d[16 * o:16 * (o + 1), :],
                              s_part3[g][32 * u:32 * u + 16, :])

    ctx.close()


def _prep_inputs(x, weight):
    """Host-side layout prep. Returns per-core input maps."""
    x = np.asarray(x, dtype=np.float32)
    weight = np.asarray(weight, dtype=np.float32)
    # bd: block-diag j-reduce, chunk cp of 8 maps f-row p -> n-row 16cp + p//8
    bd_all = np.zeros((128, 8 * 128), dtype=bfnp)
    for cp in range(8):
        for p in range(128):
            bd_all[p, 128 * cp + 16 * cp + p // 8] = 1.0
    # or2: one-hot row-replicate |s|^2 within each 32-block
    or2 = np.zeros((128, 128), dtype=bfnp)
    for u in range(4):
        or2[32 * u:32 * u + 16, 32 * u:32 * u + 16] = 1.0
    in_maps = []
    for k in range(NCORES):
        n0, n1 = NLOC * k, NLOC * (k + 1)
        xs = x[:, n0:n1, :]                      # [B, 144, 8]
        xT = np.ascontiguousarray(
            xs.transpose(1, 2, 0).reshape(F, B)).astype(bfnp)
        # x2: n-major [n, j, b] for n < 128
        x2 = np.ascontiguousarray(
            xs[:, :128, :].transpose(1, 2, 0).reshape(128, 8 * B)).astype(bfnp)
        Wk = weight[:, n0:n1, :, :]              # [10, 144, 16, 8]
        w2 = np.ascontiguousarray(
            Wk.transpose(1, 3, 0, 2).reshape(F, OI)).astype(bfnp)
        # wtp: doublerow fp8 stationary [3][128, 2, F]:
        #   row 32u+k, subtile t2, col f = WT_SCALE * W2[f, 16*(o0+u) + 2k+t2]
        w2t = w2.astype(np.float32).T            # [160, F]
        wtp = np.zeros((3, 128, 2, F), dtype=f8np)
        for g, (o0, nu) in enumerate(GROUPS):
            for u in range(nu):
                o = o0 + u
                blk = w2t[16 * o:16 * (o + 1), :]          # [16, F] (i, f)
                wtp[g, 32 * u:32 * u + 8, 0, :] = (WT_SCALE * blk[0::2]).astype(f8np)
                wtp[g, 32 * u:32 * u + 8, 1, :] = (WT_SCALE * blk[1::2]).astype(f8np)
        wtp = wtp.reshape(384, 2 * F)
        # w2j: [8][128 n, 320]: col 32o+i = W[o, n, i, j]
        w2j = np.zeros((8, 128, 320), dtype=bfnp)
        for o in range(N_OUT):
            # Wk[o,n,i,j] with n<128
            # Wk[o, n, i, j] -> w2j[j, n, i]
            w2j[:, :, 32 * o:32 * o + 16] = Wk[o, :128].transpose(2, 0, 1)
        w2j = w2j.reshape(8 * 128, 320)
        # w2pt: tail chunk (f rows 1024:1152 = n 128:144), col 32o+i
        w2pt = np.zeros((128, 320), dtype=bfnp)
        for o in range(N_OUT):
            w2pt[:, 32 * o:32 * o + 16] = w2[1024:1152, 16 * o:16 * (o + 1)]
        in_maps.append({
            "xT": xT, "x2": x2, "w2": w2, "wtp": wtp.astype(f8np),
            "w2j": w2j, "w2pt": w2pt, "bd": bd_all, "or2": or2,
        })
    return in_maps


def _squash_np(s):
    norm = np.linalg.norm(s, axis=-1, keepdims=True)
    return (norm ** 2 / (1.0 + norm ** 2) / (norm + 1e-8)) * s


def run_spmd(x, weight, trace=False, tmpdir=None):
    global _built
    if _built is None:
        _built = _build()
    nc = _built
    in_maps = _prep_inputs(x, weight)
    res = run_bass_kernel_spmd(
        nc, in_maps, list(range(NCORES)), trace=trace, tmpdir=tmpdir)
    s2 = np.zeros((OI, B), dtype=np.float32)
    for k in range(NCORES):
        s2 += res.results[k]["out"].astype(np.float32)
    s2 = s2.reshape(N_OUT, D_OUT, B).transpose(2, 0, 1)  # [B, 10, 16]
    out = _squash_np(s2).astype(np.float32)
    return out, res


def kernel(x, weight):
    out, _ = run_spmd(x, weight)
    return out


# revision 12
# speedup vs baseline: 1.2382x; 1.0109x over previous
"""DenseCapsule dynamic-routing kernel for 8 Trainium2 NeuronCores.

Strategy (contraction/n sharding, full batch per core):
  - x_hat is never materialized. All routing contractions go through the
    shared weight W on the PE:
      s[(o,i),b]   = sum_f W2[f,(o,i)] * (c (*) x)[f,b]      (f = (n,j))
      t~[o][f,b]   = sum_i W2[f,(o,i)] * (g*s)[(o,i),b]      (fp8 DoubleRow)
      b_inc[o][n,b]= sum_j x[f,b] * t~[o][f,b]               (block-diag PE)
  - Each core owns n in [144k, 144k+144); full batch B=512 rides in the
    matmul free dim.
  - s partials are AllReduced (iters 0,1); final squash on host.
  - routing logits are never materialized: c2 ~ c1 (*) exp(b_inc1), with
    exp read directly from PSUM on the ACT engine.
  - y = c (*) x runs in n-major layout so c broadcasts via a stride-0 AP
    (no DMA partition-replication); only the 16-row n-tail uses the
    DRAM-broadcast path.
"""

import sys

sys.path.insert(0, "/opt/trn_rl_repo")

import numpy as np
import ml_dtypes

import concourse.bass as bass  # noqa: F401
import concourse.tile as tile
from concourse import bacc, mybir
from concourse.bass_utils import run_bass_kernel_spmd

B, N_IN, D_IN, N_OUT, D_OUT = 512, 1152, 8, 10, 16
NCORES = 8
NLOC = N_IN // NCORES  # 144
F = NLOC * D_IN        # 1152 f-rows per core, f = 8*n_within + j
NCH = F // 128         # 9 chunks
OI = N_OUT * D_OUT     # 160
BF16 = mybir.dt.bfloat16
FP8 = mybir.dt.float8e4
F32 = mybir.dt.float32
AF = mybir.ActivationFunctionType
ALU = mybir.AluOpType
PM = mybir.MatmulPerfMode
bfnp = ml_dtypes.bfloat16
f8np = ml_dtypes.float8_e4m3fn

WT_SCALE = 64.0   # w2tp_dr stored as fp8 * WT_SCALE
GS_SCALE = 16.0   # ghat folded scale so (g*s) fp8 is well-resolved
UNSCALE = 1.0 / (WT_SCALE * GS_SCALE)  # applied in exp(b_inc)

GROUPS = ((0, 4), (4, 4), (8, 2))  # (o0, nu) per group

_built = None


def _build():
    nc = bacc.Bacc("TRN2", target_bir_lowering=False, debug=False, num_devices=NCORES)

    xT_d = nc.dram_tensor("xT", [F, B], BF16, kind="ExternalInput")
    x2_d = nc.dram_tensor("x2", [128, 8 * B], BF16, kind="ExternalInput")
    w2_d = nc.dram_tensor("w2", [F, OI], BF16, kind="ExternalInput")
    wtp_d = nc.dram_tensor("wtp", [384, 2 * F], FP8, kind="ExternalInput")
    w2j_d = nc.dram_tensor("w2j", [8 * 128, 320], BF16, kind="ExternalInput")
    w2pt_d = nc.dram_tensor("w2pt", [128, 320], BF16, kind="ExternalInput")
    bd_d = nc.dram_tensor("bd", [128, 8 * 128], BF16, kind="ExternalInput")
    or2_d = nc.dram_tensor("or2", [128, 128], BF16, kind="ExternalInput")
    out_d = nc.dram_tensor("out", [OI, B], BF16, kind="ExternalOutput")

    with tile.TileContext(nc) as tc, nc.allow_low_precision(
            reason="bf16 routing logits / fp8 agreement path within tolerance"):
        _emit(tc, nc, xT_d, x2_d, w2_d, wtp_d, w2j_d, w2pt_d, bd_d,
              or2_d, out_d)
    nc.compile()
    return nc


def _emit(tc, nc, xT_d, x2_d, w2_d, wtp_d, w2j_d, w2pt_d, bd_d,
          or2_d, out_d):
    from contextlib import ExitStack

    ctx = ExitStack()
    const = ctx.enter_context(tc.tile_pool(name="const", bufs=1))
    small = ctx.enter_context(tc.tile_pool(name="small", bufs=1))
    pairp = ctx.enter_context(tc.tile_pool(name="pair", bufs=4))
    yp = ctx.enter_context(tc.tile_pool(name="y", bufs=3))
    # PSUM tags: "pt" (t~ pairs + psos) 2x4KB, "pb" (pba/pn2/p0a) 4KB,
    # "pbt" (pbb/p0b) 4KB -> 16KB total
    psp = ctx.enter_context(tc.tile_pool(name="psp", bufs=1, space="PSUM"))
    dram = ctx.enter_context(tc.tile_pool(name="dram", bufs=1, space="DRAM"))

    # ---- load constants (priority order: s0 path first) ----
    xT = []
    for c in range(NCH):
        t = const.tile([128, B], BF16, tag=f"xT{c}", name=f"xT{c}")
        nc.sync.dma_start(t[:], xT_d[128 * c:128 * (c + 1), :])
        xT.append(t)
    w2l = []
    for c in range(NCH):
        t = const.tile([128, OI], BF16, tag=f"w2l{c}", name=f"w2l{c}")
        nc.scalar.dma_start(t[:], w2_d[128 * c:128 * (c + 1), :])
        w2l.append(t)
    wtp = []
    for g in range(3):
        t = const.tile([128, 2 * F], FP8, tag=f"wtp{g}", name=f"wtp{g}")
        (nc.sync if g % 2 else nc.scalar).dma_start(
            t[:], wtp_d[128 * g:128 * (g + 1), :])
        wtp.append(t)
    or2 = const.tile([128, 128], BF16, tag="or2", name="or2")
    nc.scalar.dma_start(or2[:], or2_d[:])
    bd = const.tile([128, 8 * 128], BF16, tag="bd", name="bd")
    nc.sync.dma_start(bd[:], bd_d[:])
    # y_s-phase constants (needed latest) on the software DGE
    x2 = const.tile([128, 8 * B], BF16, tag="x2", name="x2")
    nc.gpsimd.dma_start(x2[:], x2_d[:])
    w2j = []
    for j in range(8):
        t = const.tile([128, 320], BF16, tag=f"w2j{j}", name=f"w2j{j}")
        nc.gpsimd.dma_start(t[:], w2j_d[128 * j:128 * (j + 1), :])
        w2j.append(t)
    w2pt = const.tile([128, 320], BF16, tag="w2pt", name="w2pt")
    nc.gpsimd.dma_start(w2pt[:], w2pt_d[:])

    # ---- persistent tiles ----
    OB = N_OUT * B  # 5120
    s_red = []   # [g] [128, (t,b)] doublerow layout: row 32u+k = (o=o0+u, i=2k+t)
    sTg = []     # [g] fp8 (g*s) in the same layout
    ghat = []    # [g] [128, 512] replicated squash gain
    sq = []
    for g in range(3):
        r = small.tile([128, 2 * B], BF16, tag=f"sred{g}", name=f"sred{g}")
        nc.gpsimd.memset(r[:], 0.0)
        s_red.append(r)
        sTg.append(small.tile([128, 2 * B], FP8, tag=f"sTg{g}", name=f"sTg{g}"))
        ghat.append(small.tile([128, B], F32, tag=f"ghat{g}", name=f"ghat{g}"))
        sq.append(small.tile([128, 2 * B], BF16, tag=f"sq{g}", name=f"sq{g}"))
    e_a = small.tile([128, OB], BF16, tag="e_a", name="e_a")       # iter-1 c
    e_b = small.tile([16, OB], BF16, tag="e_b", name="e_b")
    f_a = small.tile([128, OB], BF16, tag="f_a", name="f_a")       # iter-2 c
    f_b = small.tile([16, OB], BF16, tag="f_b", name="f_b")
    za5 = small.tile([128, 5 * B], BF16, tag="za5", name="za5")
    zb5 = small.tile([16, 5 * B], BF16, tag="zb5", name="zb5")
    za2 = small.tile([128, 2 * B], BF16, tag="za2", name="za2")
    zb2 = small.tile([16, 2 * B], BF16, tag="zb2", name="zb2")
    s_part3 = [small.tile([128, B], BF16, tag=f"spart{g}", name=f"spart{g}")
               for g in range(3)]
    ln_bias = small.tile([128, 1], F32, tag="lnb", name="ln_bias")
    nc.gpsimd.memset(ln_bias[:], 1e-20)
    inva_bias = small.tile([128, 1], F32, tag="invab", name="inva_bias")
    nc.gpsimd.memset(inva_bias[:], 10.0)
    lngs_bias = small.tile([128, 1], F32, tag="lngsb", name="lngs_bias")
    nc.gpsimd.memset(lngs_bias[:], float(np.log(GS_SCALE)))

    ar_in = {t: dram.tile([OI, B], BF16, tag=f"arin{t}", name=f"arin{t}") for t in (0, 1)}
    ar_out = {t: dram.tile([OI, B], BF16, tag=f"arout{t}", name=f"arout{t}") for t in (0, 1)}
    eb_dram = [dram.tile([16, OB], BF16, tag=f"ebd{t}", name=f"ebd{t}") for t in range(2)]
    zib_dram = [dram.tile([16, B], BF16, tag=f"zibd{t}", name=f"zibd{t}") for t in range(2)]

    def sl(o):
        return slice(B * o, B * (o + 1))

    # ====== iteration 0: s0 partial = sum_{f local} W2 * x, then AllReduce ==
    p0a = psp.tile([128, B], F32, tag="pb", bufs=1, name="s0a")
    p0b = psp.tile([32, B], F32, tag="pbt", bufs=1, name="s0b")
    for c in range(NCH):
        nc.tensor.matmul(p0a[:], w2l[c][:, 0:128], xT[c][:],
                         start=(c == 0), stop=(c == NCH - 1))
    for c in range(NCH):
        nc.tensor.matmul(p0b[:], w2l[c][:, 128:160], xT[c][:],
                         start=(c == 0), stop=(c == NCH - 1))
    s0sb_a = small.tile([128, B], BF16, tag="s0sba", name="s0sba")
    s0sb_b = small.tile([32, B], BF16, tag="s0sbb", name="s0sbb")
    nc.scalar.copy(s0sb_a[:], p0a[:])
    nc.scalar.copy(s0sb_b[:], p0b[:])
    nc.sync.dma_start(ar_in[0][0:128, :], s0sb_a[:])
    nc.sync.dma_start(ar_in[0][128:160, :], s0sb_b[:])

    def allreduce(t):
        nc.gpsimd.collective_compute(
            "AllReduce", ALU.add, replica_groups=[list(range(NCORES))],
            ins=[ar_in[t].opt()], outs=[ar_out[t].opt()],
        )
        # scatter into doublerow layout: row 16o+i -> part 32u+k, free (t2,b)
        for g, (o0, nu) in enumerate(GROUPS):
            for u in range(nu):
                o = o0 + u
                src = ar_out[t][16 * o:16 * (o + 1), :].rearrange(
                    "(k t2) b -> k t2 b", k=8)
                dst = s_red[g][32 * u:32 * u + 8, :].rearrange(
                    "k (t2 b) -> k t2 b", t2=2)
                (nc.sync if (g + u) % 2 == 0 else nc.scalar).dma_start(dst, src)

    allreduce(0)

    def g_chain(it, alpha):
        """ghat[g] <- GS * alpha^2 sqrt(n2)/(1+alpha^2 n2) replicated;
        sTg[g] <- fp8 ghat*s.  All transcendentals on ACT (Ln/Exp)."""
        a2 = float(alpha * alpha)
        inva = inva_bias[:] if alpha != 1.0 else 1.0
        pn2s, l1s, l2s, diffs = [], [], [], []
        for g in range(3):
            nc.vector.tensor_mul(sq[g][:], s_red[g][:], s_red[g][:])
            pn2 = psp.tile([128, B], F32, tag="pt", bufs=2, name=f"n2_{it}_{g}")
            nc.tensor.matmul(pn2[:], or2[:], sq[g][:, 0:B], start=True, stop=False)
            nc.tensor.matmul(pn2[:], or2[:], sq[g][:, B:2 * B], start=False, stop=True)
            pn2s.append(pn2)
            l1 = small.tile([128, B], F32, tag=f"gl1{g}", name=f"gl1{it}{g}")
            nc.scalar.activation(l1[:], pn2[:], AF.Ln, scale=a2, bias=ln_bias[:])
            l2 = small.tile([128, B], F32, tag=f"gl2{g}", name=f"gl2{it}{g}")
            nc.scalar.activation(l2[:], pn2[:], AF.Ln, scale=float(alpha), bias=inva)
            l1s.append(l1)
            l2s.append(l2)
        for g in range(3):
            d = small.tile([128, B], F32, tag=f"gdf{g}", name=f"gdf{it}{g}")
            nc.vector.scalar_tensor_tensor(d[:], l1s[g][:], 0.5, l2s[g][:],
                                           op0=ALU.mult, op1=ALU.subtract)
            diffs.append(d)
        for g in range(3):
            nc.scalar.activation(ghat[g][:], diffs[g][:], AF.Exp,
                                 bias=lngs_bias[:])
        for g in range(3):
            nc.vector.tensor_mul(
                sTg[g][:].rearrange("p (t2 b) -> p t2 b", t2=2),
                ghat[g][:].unsqueeze(1).broadcast_to((128, 2, B)),
                s_red[g][:].rearrange("p (t2 b) -> p t2 b", t2=2))

    def agreement(it, e_main, e_tail):
        """e_main/e_tail <- exp(UNSCALE * b_inc) (t=0) via ACT from PSUM."""
        for g, (o0, nu) in enumerate(GROUPS):
            for pi in range(nu // 2):
                u0 = 2 * pi
                pba = psp.tile([128, 2 * B], F32, tag="pb", bufs=1, name=f"ba{it}{g}{pi}")
                pbb = psp.tile([16, 2 * B], F32, tag="pbt", bufs=1, name=f"bb{it}{g}{pi}")
                for c in range(NCH):
                    ptp = psp.tile([128, 2 * B], F32, tag="pt", bufs=2, name="pt")
                    for du in range(2):
                        u = u0 + du
                        nc.tensor.matmul(
                            ptp[:, B * du:B * (du + 1)],
                            wtp[g][32 * u:32 * u + 8, :].rearrange(
                                "k (t2 f) -> k t2 f", t2=2)[:, :, 128 * c:128 * (c + 1)],
                            sTg[g][32 * u:32 * u + 8, :].rearrange(
                                "k (t2 b) -> k t2 b", t2=2),
                            start=True, stop=True, perf_mode=PM.DoubleRow,
                            tile_position=(32 * u, 0))
                    # drain: p = t~ (*) x, alternating direct / cast+mul
                    pp = pairp.tile([128, 2 * B], BF16, tag="pp", name="pp")
                    if c % 2 == 0:
                        nc.vector.tensor_mul(
                            pp[:].rearrange("p (u b) -> p u b", u=2),
                            ptp[:].rearrange("p (u b) -> p u b", u=2),
                            xT[c][:].unsqueeze(1).broadcast_to((128, 2, B)))
                    else:
                        tsb = pairp.tile([128, 2 * B], BF16, tag="tsb", name="tsb")
                        nc.scalar.copy(tsb[:], ptp[:])
                        nc.vector.tensor_mul(
                            pp[:].rearrange("p (u b) -> p u b", u=2),
                            tsb[:].rearrange("p (u b) -> p u b", u=2),
                            xT[c][:].unsqueeze(1).broadcast_to((128, 2, B)))
                    for du in range(2):
                        if c < 8:
                            nc.tensor.matmul(pba[:, B * du:B * (du + 1)],
                                             bd[:, 128 * c:128 * (c + 1)],
                                             pp[:, B * du:B * (du + 1)],
                                             start=(c == 0), stop=(c == 7))
                        else:
                            nc.tensor.matmul(pbb[:, B * du:B * (du + 1)],
                                             bd[:, 0:16], pp[:, B * du:B * (du + 1)],
                                             start=True, stop=True)
                o = o0 + u0
                nc.scalar.activation(e_main[:, B * o:B * (o + 2)], pba[:],
                                     AF.Exp, scale=UNSCALE)
                nc.scalar.activation(e_tail[:, B * o:B * (o + 2)], pbb[:],
                                     AF.Exp, scale=UNSCALE)

    def softmax_zi(it, e_main, e_tail):
        """x-tilde fold: xt2 <- x2 * (1/sum_o e_main) broadcast over j;
        xtt <- xT[8] * (1/sum_o e_tail) f-major. e stays unnormalized."""
        zis = {}
        for (e, z5, z2, P) in ((e_main, za5, za2, 128), (e_tail, zb5, zb2, 16)):
            nc.vector.tensor_add(z5[:], e[:, 0:5 * B], e[:, 5 * B:10 * B])
            nc.vector.tensor_add(z2[:], z5[:, 0:2 * B], z5[:, 2 * B:4 * B])
            z = small.tile([P, B], F32, tag=f"z{P}", name=f"z{P}")
            nc.vector.tensor_add(z[:], z2[:, 0:B], z2[:, B:2 * B])
            nc.vector.tensor_add(z[:], z[:], z5[:, 4 * B:5 * B])
            zf = small.tile([P, B], F32, tag=f"zf{P}", name=f"zf{P}")
            nc.vector.reciprocal_approx_fast(zf[:], z[:])
            zi = small.tile([P, B], BF16, tag=f"zi{P}", name=f"zi{P}")
            nc.vector.tensor_copy(zi[:], zf[:])
            zis[P] = zi
        xt2 = small.tile([128, 8 * B], BF16, tag="xt2", name="xt2")
        nc.vector.tensor_mul(
            xt2[:].rearrange("p (j b) -> p j b", j=8),
            x2[:].rearrange("p (j b) -> p j b", j=8),
            zis[128][:].unsqueeze(1).broadcast_to((128, 8, B)))
        nc.sync.dma_start(zib_dram[it][:], zis[16][:])
        zibf = small.tile([128, B], BF16, tag="zibf", name="zibf")
        nc.sync.dma_start(
            zibf[:], zib_dram[it][:, :].unsqueeze(1).broadcast_to((16, 8, B)))
        xtt = small.tile([128, B], BF16, tag="xtt", name="xtt")
        nc.vector.tensor_mul(xtt[:], xT[8][:], zibf[:])
        return xt2, xtt

    def y_s_phase(it, e_main, e_tail, xt2, xtt):
        """s_part3[g] rows 32u:+16 <- sum_f W2[f,(o,:)]*(c (*) x)[f,:], o=o0+u."""
        nc.sync.dma_start(eb_dram[it][:], e_tail[:])
        psos = []
        for g, (o0, nu) in enumerate(GROUPS):
            ps = psp.tile([128, B], F32, tag="pt", bufs=2, name=f"so{it}{g}")
            psos.append(ps)
            for u in range(nu):
                o = o0 + u
                y2 = yp.tile([128, 8 * B], BF16, tag="y2", name="y2")
                nc.vector.tensor_mul(
                    y2[:].rearrange("p (j b) -> p j b", j=8),
                    xt2[:].rearrange("p (j b) -> p j b", j=8),
                    e_main[:, sl(o)].unsqueeze(1).broadcast_to((128, 8, B)))
                for j in range(8):
                    nc.tensor.matmul(ps[32 * u:32 * (u + 1), :],
                                     w2j[j][:, 32 * o:32 * (o + 1)],
                                     y2[:, B * j:B * (j + 1)],
                                     start=(j == 0), stop=False,
                                     tile_position=(0, 32 * u))
                cxt = yp.tile([128, B], BF16, tag="cxt", name="cxt")
                nc.sync.dma_start(
                    cxt[:],
                    eb_dram[it][:, sl(o)].unsqueeze(1).broadcast_to((16, 8, B)))
                yt = yp.tile([128, B], BF16, tag="yt", name="yt")
                nc.vector.tensor_mul(yt[:], xtt[:], cxt[:])
                nc.tensor.matmul(ps[32 * u:32 * (u + 1), :],
                                 w2pt[:, 32 * o:32 * (o + 1)], yt[:],
                                 start=False, stop=True,
                                 tile_position=(0, 32 * u))
            nc.scalar.copy(s_part3[g][:], psos[g][:])
            if it == 0:
                for u in range(nu):
                    o = o0 + u
                    nc.sync.dma_start(ar_in[1][16 * o:16 * (o + 1), :],
                                      s_part3[g][32 * u:32 * u + 16, :])

    # =====================  routing  =====================================
    g_chain(0, 0.1)
    agreement(0, e_a, e_b)
    xt2_0, xtt_0 = softmax_zi(0, e_a, e_b)
    y_s_phase(0, e_a, e_b, xt2_0, xtt_0)

    # ---- AllReduce s1 (staging DMAs issued per-group inside y_s_phase) ----
    allreduce(1)

    g_chain(1, 1.0)
    agreement(1, f_a, f_b)
    # c2 ~ e1 (*) exp(b_inc1) (unnormalized), renormalized via x-tilde
    nc.vector.tensor_mul(f_a[:], f_a[:], e_a[:])
    nc.vector.tensor_mul(f_b[:], f_b[:], e_b[:])
    xt2_1, xtt_1 = softmax_zi(1, f_a, f_b)
    y_s_phase(1, f_a, f_b, xt2_1, xtt_1)

    # ---- write s2 partials ----
    for g, (o0, nu) in enumerate(GROUPS):
        for u in range(nu):
            o = o0 + u
            nc.sync.dma_start(out_d[16 * o:16 * (o + 1), :],
                              s_part3[g][32 * u:32 * u + 16, :])

    ctx.close()


def _prep_inputs(x, weight):
    """Host-side layout prep. Returns per-core input maps."""
    x = np.asarray(x, dtype=np.float32)
    weight = np.asarray(weight, dtype=np.float32)
    # bd: block-diag j-reduce, chunk cp of 8 maps f-row p -> n-row 16cp + p//8
    bd_all = np.zeros((128, 8 * 128), dtype=bfnp)
    for cp in range(8):
        for p in range(128):
            bd_all[p, 128 * cp + 16 * cp + p // 8] = 1.0
    # or2: one-hot row-replicate |s|^2 within each 32-block
    or2 = np.zeros((128, 128), dtype=bfnp)
    for u in range(4):
        or2[32 * u:32 * u + 16, 32 * u:32 * u + 16] = 1.0
    in_maps = []
    for k in range(NCORES):
        n0, n1 = NLOC * k, NLOC * (k + 1)
        xs = x[:, n0:n1, :]                      # [B, 144, 8]
        xT = np.ascontiguousarray(
            xs.transpose(1, 2, 0).reshape(F, B)).astype(bfnp)
        # x2: n-major [n, j, b] for n < 128
        x2 = np.ascontiguousarray(
            xs[:, :128, :].transpose(1, 2, 0).reshape(128, 8 * B)).astype(bfnp)
        Wk = weight[:, n0:n1, :, :]              # [10, 144, 16, 8]
        w2 = np.ascontiguousarray(
            Wk.transpose(1, 3, 0, 2).reshape(F, OI)).astype(bfnp)
        # wtp: doublerow fp8 stationary [3][128, 2, F]:
        #   row 32u+k, subtile t2, col f = WT_SCALE * W2[f, 16*(o0+u) + 2k+t2]
        w2t = w2.astype(np.float32).T            # [160, F]
        wtp = np.zeros((3, 128, 2, F), dtype=f8np)
        for g, (o0, nu) in enumerate(GROUPS):
            for u in range(nu):
                o = o0 + u
                blk = w2t[16 * o:16 * (o + 1), :]          # [16, F] (i, f)
                wtp[g, 32 * u:32 * u + 8, 0, :] = (WT_SCALE * blk[0::2]).astype(f8np)
                wtp[g, 32 * u:32 * u + 8, 1, :] = (WT_SCALE * blk[1::2]).astype(f8np)
        wtp = wtp.reshape(384, 2 * F)
        # w2j: [8][128 n, 320]: col 32o+i = W[o, n, i, j]
        w2j = np.zeros((8, 128, 320), dtype=bfnp)
        for o in range(N_OUT):
            # Wk[o,n,i,j] with n<128
            # Wk[o, n, i, j] -> w2j[j, n, i]
            w2j[:, :, 32 * o:32 * o + 16] = Wk[o, :128].transpose(2, 0, 1)
        w2j = w2j.reshape(8 * 128, 320)
        # w2pt: tail chunk (f rows 1024:1152 = n 128:144), col 32o+i
        w2pt = np.zeros((128, 320), dtype=bfnp)
        for o in range(N_OUT):
            w2pt[:, 32 * o:32 * o + 16] = w2[1024:1152, 16 * o:16 * (o + 1)]
        in_maps.append({
            "xT": xT, "x2": x2, "w2": w2, "wtp": wtp.astype(f8np),
            "w2j": w2j, "w2pt": w2pt, "bd": bd_all, "or2": or2,
        })
    return in_maps


def _squash_np(s):
    norm = np.linalg.norm(s, axis=-1, keepdims=True)
    return (norm ** 2 / (1.0 + norm ** 2) / (norm + 1e-8)) * s


def run_spmd(x, weight, trace=False, tmpdir=None):
    global _built
    if _built is None:
        _built = _build()
    nc = _built
    in_maps = _prep_inputs(x, weight)
    res = run_bass_kernel_spmd(
        nc, in_maps, list(range(NCORES)), trace=trace, tmpdir=tmpdir)
    s2 = np.zeros((OI, B), dtype=np.float32)
    for k in range(NCORES):
        s2 += res.results[k]["out"].astype(np.float32)
    s2 = s2.reshape(N_OUT, D_OUT, B).transpose(2, 0, 1)  # [B, 10, 16]
    out = _squash_np(s2).astype(np.float32)
    return out, res


def kernel(x, weight):
    out, _ = run_spmd(x, weight)
    return out


# revision 19
# speedup vs baseline: 1.2447x; 1.0053x over previous
"""DenseCapsule dynamic-routing kernel for 8 Trainium2 NeuronCores.

Strategy (contraction/n sharding, full batch per core):
  - x_hat is never materialized. All routing contractions go through the
    shared weight W on the PE:
      s[(o,i),b]   = sum_f W2[f,(o,i)] * (c (*) x)[f,b]      (f = (n,j))
      t~[o][f,b]   = sum_i W2[f,(o,i)] * (g*s)[(o,i),b]      (fp8 DoubleRow)
      b_inc[o][n,b]= sum_j x[f,b] * t~[o][f,b]               (block-diag PE)
  - Each core owns n in [144k, 144k+144); full batch B=512 rides in the
    matmul free dim.
  - s partials are AllReduced (iters 0,1); final squash on host.
  - routing logits are never materialized: c2 ~ c1 (*) exp(b_inc1), with
    exp read directly from PSUM on the ACT engine.
  - y = c (*) x runs in n-major layout so c broadcasts via a stride-0 AP
    (no DMA partition-replication); only the 16-row n-tail uses the
    DRAM-broadcast path.
"""

import sys

sys.path.insert(0, "/opt/trn_rl_repo")

import numpy as np
import ml_dtypes

import concourse.bass as bass  # noqa: F401
import concourse.tile as tile
from concourse import bacc, mybir
from concourse.bass_utils import run_bass_kernel_spmd

B, N_IN, D_IN, N_OUT, D_OUT = 512, 1152, 8, 10, 16
NCORES = 8
NLOC = N_IN // NCORES  # 144
F = NLOC * D_IN        # 1152 f-rows per core, f = 8*n_within + j
NCH = F // 128         # 9 chunks
OI = N_OUT * D_OUT     # 160
BF16 = mybir.dt.bfloat16
FP8 = mybir.dt.float8e4
F32 = mybir.dt.float32
AF = mybir.ActivationFunctionType
ALU = mybir.AluOpType
PM = mybir.MatmulPerfMode
bfnp = ml_dtypes.bfloat16
f8np = ml_dtypes.float8_e4m3fn

WT_SCALE = 64.0   # w2tp_dr stored as fp8 * WT_SCALE
GS_SCALE = 16.0   # ghat folded scale so (g*s) fp8 is well-resolved
UNSCALE = 1.0 / (WT_SCALE * GS_SCALE)  # applied in exp(b_inc)

GROUPS = ((0, 4), (4, 4), (8, 2))  # (o0, nu) per group

_built = None


def _build():
    nc = bacc.Bacc("TRN2", target_bir_lowering=False, debug=False, num_devices=NCORES)

    xT_d = nc.dram_tensor("xT", [F, B], BF16, kind="ExternalInput")
    x2_d = nc.dram_tensor("x2", [128, 8 * B], BF16, kind="ExternalInput")
    xdr_d = nc.dram_tensor("xdr", [128, 36 * 2 * B], FP8, kind="ExternalInput")
    wdr_d = nc.dram_tensor("wdr", [128, 36 * 320], FP8, kind="ExternalInput")
    wtp_d = nc.dram_tensor("wtp", [384, 2 * F], FP8, kind="ExternalInput")
    w2j_d = nc.dram_tensor("w2j", [8 * 128, 320], BF16, kind="ExternalInput")
    w2pt_d = nc.dram_tensor("w2pt", [128, 320], BF16, kind="ExternalInput")
    bd_d = nc.dram_tensor("bd", [128, 8 * 128], BF16, kind="ExternalInput")
    or2_d = nc.dram_tensor("or2", [128, 128], BF16, kind="ExternalInput")
    out_d = nc.dram_tensor("out", [OI, B], BF16, kind="ExternalOutput")

    with tile.TileContext(nc) as tc, nc.allow_low_precision(
            reason="bf16 routing logits / fp8 agreement path within tolerance"):
        _emit(tc, nc, xT_d, x2_d, xdr_d, wdr_d, wtp_d, w2j_d, w2pt_d, bd_d,
              or2_d, out_d)
    nc.compile()
    return nc


def _emit(tc, nc, xT_d, x2_d, xdr_d, wdr_d, wtp_d, w2j_d, w2pt_d, bd_d,
          or2_d, out_d):
    from contextlib import ExitStack

    ctx = ExitStack()
    const = ctx.enter_context(tc.tile_pool(name="const", bufs=1))
    small = ctx.enter_context(tc.tile_pool(name="small", bufs=1))
    pairp = ctx.enter_context(tc.tile_pool(name="pair", bufs=3))
    yp = ctx.enter_context(tc.tile_pool(name="y", bufs=2))
    # PSUM tags: "pt" (t~ pairs + psos) 2x4KB, "pb" (pba/pn2/p0a) 4KB,
    # "pbt" (pbb/p0b) 4KB -> 16KB total
    psp = ctx.enter_context(tc.tile_pool(name="psp", bufs=1, space="PSUM"))
    dram = ctx.enter_context(tc.tile_pool(name="dram", bufs=1, space="DRAM"))

    # ---- collective warmup: inits the CC stream for the single real AR ----
    wu_in = dram.tile([16, 16], F32, tag="wu_in", name="wu_in")
    wu_out = dram.tile([16, 16], F32, tag="wu_out", name="wu_out")
    nc.gpsimd.collective_compute(
        "AllReduce", ALU.add, replica_groups=[list(range(NCORES))],
        ins=[wu_in.opt()], outs=[wu_out.opt()],
    )

    # ---- load constants (priority order: s0 path first) ----
    xT = []
    for c in range(NCH):
        t = const.tile([128, B], BF16, tag=f"xT{c}", name=f"xT{c}")
        nc.sync.dma_start(t[:], xT_d[128 * c:128 * (c + 1), :])
        xT.append(t)
    xdr = const.tile([128, 36 * 2 * B], FP8, tag="xdr", name="xdr")
    nc.scalar.dma_start(xdr[:], xdr_d[:])
    wdr = const.tile([128, 36 * 320], FP8, tag="wdr", name="wdr")
    nc.scalar.dma_start(wdr[:], wdr_d[:])
    wtp = []
    for g in range(3):
        t = const.tile([128, 2 * F], FP8, tag=f"wtp{g}", name=f"wtp{g}")
        (nc.sync if g % 2 else nc.scalar).dma_start(
            t[:], wtp_d[128 * g:128 * (g + 1), :])
        wtp.append(t)
    or2 = const.tile([128, 128], BF16, tag="or2", name="or2")
    nc.scalar.dma_start(or2[:], or2_d[:])
    bd = const.tile([128, 8 * 128], BF16, tag="bd", name="bd")
    nc.sync.dma_start(bd[:], bd_d[:])
    # y_s-phase constants (needed latest) on the software DGE
    x2 = const.tile([128, 8 * B], BF16, tag="x2", name="x2")
    nc.gpsimd.dma_start(x2[:], x2_d[:])
    w2j = []
    for j in range(8):
        t = const.tile([128, 320], BF16, tag=f"w2j{j}", name=f"w2j{j}")
        nc.gpsimd.dma_start(t[:], w2j_d[128 * j:128 * (j + 1), :])
        w2j.append(t)
    w2pt = const.tile([128, 320], BF16, tag="w2pt", name="w2pt")
    nc.gpsimd.dma_start(w2pt[:], w2pt_d[:])

    # ---- persistent tiles ----
    OB = N_OUT * B  # 5120
    s_red = []   # [g] [128, (t,b)] doublerow layout: row 32u+k = (o=o0+u, i=2k+t)
    sTg = []     # [g] fp8 (g*s) in the same layout
    ghat = []    # [g] [128, 512] replicated squash gain
    for g in range(3):
        r = small.tile([128, 2 * B], BF16, tag=f"sred{g}", name=f"sred{g}")
        nc.gpsimd.memset(r[:], 0.0)
        s_red.append(r)
        sTg.append(small.tile([128, 2 * B], FP8, tag=f"sTg{g}", name=f"sTg{g}"))
        ghat.append(small.tile([128, B], BF16, tag=f"ghat{g}", name=f"ghat{g}"))
    e_a = small.tile([128, OB], BF16, tag="e_a", name="e_a")       # iter-1 c
    f_a = small.tile([128, OB], BF16, tag="f_a", name="f_a")       # iter-2 c
    e_b = small.tile([16, OB], BF16, tag="e_b", name="e_b")
    f_b = small.tile([16, OB], BF16, tag="f_b", name="f_b")
    za2 = small.tile([128, 3 * B], BF16, tag="za2", name="za2")
    zb2 = small.tile([16, 3 * B], BF16, tag="zb2", name="zb2")
    s_part3 = [small.tile([128, B], BF16, tag=f"spart{g}", name=f"spart{g}")
               for g in range(3)]
    ln_bias = small.tile([128, 1], F32, tag="lnb", name="ln_bias")
    nc.gpsimd.memset(ln_bias[:], 1e-20)
    inva_bias = small.tile([128, 1], F32, tag="invab", name="inva_bias")
    nc.gpsimd.memset(inva_bias[:], 10.0)
    lngs_bias = small.tile([128, 1], F32, tag="lngsb", name="lngs_bias")
    nc.gpsimd.memset(lngs_bias[:], float(np.log(GS_SCALE)))

    ar_in = {1: dram.tile([OI, B], BF16, tag="arin1", name="arin1")}
    ar_out = {1: dram.tile([OI, B], BF16, tag="arout1", name="arout1")}
    eb_dram = [dram.tile([16, OB], BF16, tag=f"ebd{t}", name=f"ebd{t}") for t in range(2)]
    zib_dram = [dram.tile([16, B], BF16, tag=f"zibd{t}", name=f"zibd{t}") for t in range(2)]

    def sl(o):
        return slice(B * o, B * (o + 1))

    # ====== iteration 0: FULL s0 computed locally on every core (no AR) ====
    p0a = psp.tile([128, B], F32, tag="pb", bufs=1, name="s0a")
    p0b = psp.tile([32, B], F32, tag="pbt", bufs=1, name="s0b")
    for cp in range(36):
        wsl = wdr[:, 320 * cp:320 * (cp + 1)].rearrange(
            "p (t2 o) -> p t2 o", t2=2)
        xsl = xdr[:, 1024 * cp:1024 * (cp + 1)].rearrange(
            "p (t2 b) -> p t2 b", t2=2)
        nc.tensor.matmul(p0a[:], wsl[:, :, 0:128], xsl,
                         start=(cp == 0), stop=(cp == 35),
                         perf_mode=PM.DoubleRow, tile_position=(0, 0))
    for cp in range(36):
        wsl = wdr[:, 320 * cp:320 * (cp + 1)].rearrange(
            "p (t2 o) -> p t2 o", t2=2)
        xsl = xdr[:, 1024 * cp:1024 * (cp + 1)].rearrange(
            "p (t2 b) -> p t2 b", t2=2)
        nc.tensor.matmul(p0b[:], wsl[:, :, 128:160], xsl,
                         start=(cp == 0), stop=(cp == 35),
                         perf_mode=PM.DoubleRow, tile_position=(0, 0))
    s0sb_a = small.tile([128, B], BF16, tag="s0sba", name="s0sba")
    s0sb_b = small.tile([32, B], BF16, tag="s0sbb", name="s0sbb")
    nc.scalar.activation(s0sb_a[:], p0a[:], AF.Copy, scale=float(1.0 / WT_SCALE))
    nc.scalar.activation(s0sb_b[:], p0b[:], AF.Copy, scale=float(1.0 / WT_SCALE))
    s0_dram = dram.tile([OI, B], BF16, tag="s0d", name="s0d")
    nc.sync.dma_start(s0_dram[0:128, :], s0sb_a[:])
    nc.scalar.dma_start(s0_dram[128:160, :], s0sb_b[:])

    def scatter_sred(src_dram):
        for g, (o0, nu) in enumerate(GROUPS):
            for u in range(nu):
                o = o0 + u
                sr = src_dram[16 * o:16 * (o + 1), :].rearrange(
                    "(k t2) b -> k t2 b", k=8)
                dst = s_red[g][32 * u:32 * u + 8, :].rearrange(
                    "k (t2 b) -> k t2 b", t2=2)
                (nc.sync if (g + u) % 2 == 0 else nc.scalar).dma_start(dst, sr)

    scatter_sred(s0_dram)

    def allreduce(t):
        nc.gpsimd.collective_compute(
            "AllReduce", ALU.add, replica_groups=[list(range(NCORES))],
            ins=[ar_in[t].opt()], outs=[ar_out[t].opt()],
        )
        scatter_sred(ar_out[t])

    def g_chain(it, alpha):
        """ghat[g] <- GS * alpha^2 sqrt(n2)/(1+alpha^2 n2) replicated;
        sTg[g] <- fp8 ghat*s.  All transcendentals on ACT (Ln/Exp)."""
        a2 = float(alpha * alpha)
        inva = inva_bias[:] if alpha != 1.0 else 1.0
        pn2s, l1s, l2s = [], [], []
        for g in range(3):
            sq = pairp.tile([128, 2 * B], BF16, tag="pp", name=f"sq{it}{g}")
            nc.vector.tensor_mul(sq[:], s_red[g][:], s_red[g][:])
            pn2 = psp.tile([128, B], F32, tag="pt", bufs=2, name=f"n2_{it}_{g}")
            nc.tensor.matmul(pn2[:], or2[:], sq[:, 0:B], start=True, stop=False)
            nc.tensor.matmul(pn2[:], or2[:], sq[:, B:2 * B], start=False, stop=True)
            pn2s.append(pn2)
            l1 = small.tile([128, B], BF16, tag=f"gl1{g}", name=f"gl1{it}{g}")
            nc.scalar.activation(l1[:], pn2[:], AF.Ln, scale=a2, bias=ln_bias[:])
            l2 = small.tile([128, B], BF16, tag=f"gl2{g}", name=f"gl2{it}{g}")
            nc.scalar.activation(l2[:], pn2[:], AF.Ln, scale=float(alpha), bias=inva)
            l1s.append(l1)
            l2s.append(l2)
        for g in range(3):
            nc.vector.scalar_tensor_tensor(l1s[g][:], l1s[g][:], 0.5, l2s[g][:],
                                           op0=ALU.mult, op1=ALU.subtract)
        for g in range(3):
            nc.scalar.activation(ghat[g][:], l1s[g][:], AF.Exp,
                                 bias=lngs_bias[:])
        for g in range(3):
            nc.vector.tensor_mul(
                sTg[g][:].rearrange("p (t2 b) -> p t2 b", t2=2),
                ghat[g][:].unsqueeze(1).broadcast_to((128, 2, B)),
                s_red[g][:].rearrange("p (t2 b) -> p t2 b", t2=2))

    def agreement(it, e_main, e_tail):
        """e_main/e_tail <- exp(UNSCALE * b_inc) (t=0) via ACT from PSUM."""
        for g, (o0, nu) in enumerate(GROUPS):
            for pi in range(nu // 2):
                u0 = 2 * pi
                pba = psp.tile([128, 2 * B], F32, tag="pb", bufs=1, name=f"ba{it}{g}{pi}")
                pbb = psp.tile([16, 2 * B], F32, tag="pbt", bufs=1, name=f"bb{it}{g}{pi}")
                for c in range(NCH):
                    ptp = psp.tile([128, 2 * B], F32, tag="pt", bufs=2, name="pt")
                    for du in range(2):
                        u = u0 + du
                        nc.tensor.matmul(
                            ptp[:, B * du:B * (du + 1)],
                            wtp[g][32 * u:32 * u + 8, :].rearrange(
                                "k (t2 f) -> k t2 f", t2=2)[:, :, 128 * c:128 * (c + 1)],
                            sTg[g][32 * u:32 * u + 8, :].rearrange(
                                "k (t2 b) -> k t2 b", t2=2),
                            start=True, stop=True, perf_mode=PM.DoubleRow,
                            tile_position=(32 * u, 0))
                    # drain: p = t~ (*) x, alternating direct / cast+mul
                    pp = pairp.tile([128, 2 * B], BF16, tag="pp", name="pp")
                    if c % 2 == 0:
                        nc.vector.tensor_mul(
                            pp[:].rearrange("p (u b) -> p u b", u=2),
                            ptp[:].rearrange("p (u b) -> p u b", u=2),
                            xT[c][:].unsqueeze(1).broadcast_to((128, 2, B)))
                    else:
                        tsb = pairp.tile([128, 2 * B], BF16, tag="tsb", name="tsb")
                        nc.scalar.copy(tsb[:], ptp[:])
                        nc.vector.tensor_mul(
                            pp[:].rearrange("p (u b) -> p u b", u=2),
                            tsb[:].rearrange("p (u b) -> p u b", u=2),
                            xT[c][:].unsqueeze(1).broadcast_to((128, 2, B)))
                    for du in range(2):
                        if c < 8:
                            nc.tensor.matmul(pba[:, B * du:B * (du + 1)],
                                             bd[:, 128 * c:128 * (c + 1)],
                                             pp[:, B * du:B * (du + 1)],
                                             start=(c == 0), stop=(c == 7))
                        else:
                            nc.tensor.matmul(pbb[:, B * du:B * (du + 1)],
                                             bd[:, 0:16], pp[:, B * du:B * (du + 1)],
                                             start=True, stop=True)
                o = o0 + u0
                nc.scalar.activation(e_main[:, B * o:B * (o + 2)], pba[:],
                                     AF.Exp, scale=UNSCALE)
                nc.scalar.activation(e_tail[:, B * o:B * (o + 2)], pbb[:],
                                     AF.Exp, scale=UNSCALE)

    def softmax_zi(it, e_main, e_tail):
        """x-tilde fold: xt2 <- x2 * (1/sum_o e_main) broadcast over j;
        xtt <- xT[8] * (1/sum_o e_tail) f-major. e stays unnormalized."""
        zis = {}
        for (e, z2, P) in ((e_main, za2, 128), (e_tail, zb2, 16)):
            # per-group partial sums so z pipelines behind each group's exps
            for g, (o0, nu) in enumerate(GROUPS):
                if nu == 4:
                    z5 = pairp.tile([P, 2 * B], BF16, tag="pp",
                                    name=f"z5{it}{P}{g}")
                    nc.vector.tensor_add(z5[:],
                                         e[:, o0 * B:(o0 + 2) * B],
                                         e[:, (o0 + 2) * B:(o0 + 4) * B])
                    nc.vector.tensor_add(z2[:, g * B:(g + 1) * B],
                                         z5[:, 0:B], z5[:, B:2 * B])
                else:
                    nc.vector.tensor_add(z2[:, g * B:(g + 1) * B],
                                         e[:, o0 * B:(o0 + 1) * B],
                                         e[:, (o0 + 1) * B:(o0 + 2) * B])
            z = small.tile([P, B], F32, tag=f"z{P}", name=f"z{P}")
            nc.vector.tensor_add(z[:], z2[:, 0:B], z2[:, B:2 * B])
            nc.vector.tensor_add(z[:], z[:], z2[:, 2 * B:3 * B])
            zf = small.tile([P, B], F32, tag=f"zf{P}", name=f"zf{P}")
            nc.vector.reciprocal_approx_fast(zf[:], z[:])
            zi = small.tile([P, B], BF16, tag=f"zi{P}", name=f"zi{P}")
            nc.vector.tensor_copy(zi[:], zf[:])
            zis[P] = zi
        xt2 = small.tile([128, 8 * B], BF16, tag="xt2", name="xt2")
        nc.vector.tensor_mul(
            xt2[:].rearrange("p (j b) -> p j b", j=8),
            x2[:].rearrange("p (j b) -> p j b", j=8),
            zis[128][:].unsqueeze(1).broadcast_to((128, 8, B)))
        nc.sync.dma_start(zib_dram[it][:], zis[16][:])
        zibf = small.tile([128, B], BF16, tag="zibf", name="zibf")
        nc.sync.dma_start(
            zibf[:], zib_dram[it][:, :].unsqueeze(1).broadcast_to((16, 8, B)))
        xtt = small.tile([128, B], BF16, tag="xtt", name="xtt")
        nc.vector.tensor_mul(xtt[:], xT[8][:], zibf[:])
        return xt2, xtt

    def y_s_phase(it, e_main, e_tail, xt2, xtt):
        """s_part3[g] rows 32u:+16 <- sum_f W2[f,(o,:)]*(c (*) x)[f,:], o=o0+u."""
        nc.sync.dma_start(eb_dram[it][:], e_tail[:])
        psos = []
        for g, (o0, nu) in enumerate(GROUPS):
            ps = psp.tile([128, B], F32, tag="pt", bufs=2, name=f"so{it}{g}")
            psos.append(ps)
            for u in range(nu):
                o = o0 + u
                y2 = yp.tile([128, 8 * B], BF16, tag="y2", name="y2")
                nc.vector.tensor_mul(
                    y2[:].rearrange("p (j b) -> p j b", j=8),
                    xt2[:].rearrange("p (j b) -> p j b", j=8),
                    e_main[:, sl(o)].unsqueeze(1).broadcast_to((128, 8, B)))
                for j in range(8):
                    nc.tensor.matmul(ps[32 * u:32 * (u + 1), :],
                                     w2j[j][:, 32 * o:32 * (o + 1)],
                                     y2[:, B * j:B * (j + 1)],
                                     start=(j == 0), stop=False,
                                     tile_position=(0, 32 * u))
                cxt = yp.tile([128, B], BF16, tag="cxt", name="cxt")
                nc.sync.dma_start(
                    cxt[:],
                    eb_dram[it][:, sl(o)].unsqueeze(1).broadcast_to((16, 8, B)))
                yt = yp.tile([128, B], BF16, tag="yt", name="yt")
                nc.vector.tensor_mul(yt[:], xtt[:], cxt[:])
                nc.tensor.matmul(ps[32 * u:32 * (u + 1), :],
                                 w2pt[:, 32 * o:32 * (o + 1)], yt[:],
                                 start=False, stop=True,
                                 tile_position=(0, 32 * u))
            nc.scalar.copy(s_part3[g][:], psos[g][:])
            if it == 0:
                for u in range(nu):
                    o = o0 + u
                    nc.sync.dma_start(ar_in[1][16 * o:16 * (o + 1), :],
                                      s_part3[g][32 * u:32 * u + 16, :])

    # =====================  routing  =====================================
    g_chain(0, 0.1)
    agreement(0, e_a, e_b)
    xt2_0, xtt_0 = softmax_zi(0, e_a, e_b)
    y_s_phase(0, e_a, e_b, xt2_0, xtt_0)

    # ---- AllReduce s1 (staging DMAs issued per-group inside y_s_phase) ----
    allreduce(1)

    g_chain(1, 1.0)
    agreement(1, f_a, f_b)
    # c2 ~ e1 (*) exp(b_inc1) (unnormalized), renormalized via x-tilde
    nc.vector.tensor_mul(f_a[:], f_a[:], e_a[:])
    nc.vector.tensor_mul(f_b[:], f_b[:], e_b[:])
    xt2_1, xtt_1 = softmax_zi(1, f_a, f_b)
    y_s_phase(1, f_a, f_b, xt2_1, xtt_1)

    # ---- write s2 partials ----
    for g, (o0, nu) in enumerate(GROUPS):
        for u in range(nu):
            o = o0 + u
            nc.sync.dma_start(out_d[16 * o:16 * (o + 1), :],
                              s_part3[g][32 * u:32 * u + 16, :])

    ctx.close()


def _prep_inputs(x, weight):
    """Host-side layout prep. Returns per-core input maps."""
    x = np.asarray(x, dtype=np.float32)
    weight = np.asarray(weight, dtype=np.float32)
    # bd: block-diag j-reduce, chunk cp of 8 maps f-row p -> n-row 16cp + p//8
    bd_all = np.zeros((128, 8 * 128), dtype=bfnp)
    for cp in range(8):
        for p in range(128):
            bd_all[p, 128 * cp + 16 * cp + p // 8] = 1.0
    # or2: one-hot row-replicate |s|^2 within each 32-block
    or2 = np.zeros((128, 128), dtype=bfnp)
    for u in range(4):
        or2[32 * u:32 * u + 16, 32 * u:32 * u + 16] = 1.0
    # full-x / full-W paired-chunk doublerow operands (same for all cores):
    # row r, col 1024*cp + 512*t2 + b  <->  xTf[128*(2cp+t2)+r, b]
    xTf = np.ascontiguousarray(
        x.transpose(1, 2, 0).reshape(N_IN * D_IN, B)).astype(np.float32)
    w2f = np.ascontiguousarray(
        weight.transpose(1, 3, 0, 2).reshape(N_IN * D_IN, OI)).astype(np.float32)
    xdr_full = np.ascontiguousarray(
        xTf.reshape(36, 2, 128, B).transpose(2, 0, 1, 3).reshape(128, 36 * 2 * B)
    ).astype(f8np)
    wdr_full = np.ascontiguousarray(
        (WT_SCALE * w2f).reshape(36, 2, 128, OI).transpose(2, 0, 1, 3).reshape(
            128, 36 * 2 * OI)).astype(f8np)
    in_maps = []
    for k in range(NCORES):
        n0, n1 = NLOC * k, NLOC * (k + 1)
        xs = x[:, n0:n1, :]                      # [B, 144, 8]
        xT = np.ascontiguousarray(
            xs.transpose(1, 2, 0).reshape(F, B)).astype(bfnp)
        # x2: n-major [n, j, b] for n < 128
        x2 = np.ascontiguousarray(
            xs[:, :128, :].transpose(1, 2, 0).reshape(128, 8 * B)).astype(bfnp)
        Wk = weight[:, n0:n1, :, :]              # [10, 144, 16, 8]
        w2 = np.ascontiguousarray(
            Wk.transpose(1, 3, 0, 2).reshape(F, OI)).astype(bfnp)
        # wtp: doublerow fp8 stationary [3][128, 2, F]:
        #   row 32u+k, subtile t2, col f = WT_SCALE * W2[f, 16*(o0+u) + 2k+t2]
        w2t = w2.astype(np.float32).T            # [160, F]
        wtp = np.zeros((3, 128, 2, F), dtype=f8np)
        for g, (o0, nu) in enumerate(GROUPS):
            for u in range(nu):
                o = o0 + u
                blk = w2t[16 * o:16 * (o + 1), :]          # [16, F] (i, f)
                wtp[g, 32 * u:32 * u + 8, 0, :] = (WT_SCALE * blk[0::2]).astype(f8np)
                wtp[g, 32 * u:32 * u + 8, 1, :] = (WT_SCALE * blk[1::2]).astype(f8np)
        wtp = wtp.reshape(384, 2 * F)
        # w2j: [8][128 n, 320]: col 32o+i = W[o, n, i, j]
        w2j = np.zeros((8, 128, 320), dtype=bfnp)
        for o in range(N_OUT):
            # Wk[o,n,i,j] with n<128
            # Wk[o, n, i, j] -> w2j[j, n, i]
            w2j[:, :, 32 * o:32 * o + 16] = Wk[o, :128].transpose(2, 0, 1)
        w2j = w2j.reshape(8 * 128, 320)
        # w2pt: tail chunk (f rows 1024:1152 = n 128:144), col 32o+i
        w2pt = np.zeros((128, 320), dtype=bfnp)
        for o in range(N_OUT):
            w2pt[:, 32 * o:32 * o + 16] = w2[1024:1152, 16 * o:16 * (o + 1)]
        in_maps.append({
            "xT": xT, "x2": x2, "xdr": xdr_full, "wdr": wdr_full,
            "wtp": wtp.astype(f8np),
            "w2j": w2j, "w2pt": w2pt, "bd": bd_all, "or2": or2,
        })
    return in_maps


def _squash_np(s):
    norm = np.linalg.norm(s, axis=-1, keepdims=True)
    return (norm ** 2 / (1.0 + norm ** 2) / (norm + 1e-8)) * s


def run_spmd(x, weight, trace=False, tmpdir=None):
    global _built
    if _built is None:
        _built = _build()
    nc = _built
    in_maps = _prep_inputs(x, weight)
    res = run_bass_kernel_spmd(
        nc, in_maps, list(range(NCORES)), trace=trace, tmpdir=tmpdir)
    s2 = np.zeros((OI, B), dtype=np.float32)
    for k in range(NCORES):
        s2 += res.results[k]["out"].astype(np.float32)
    s2 = s2.reshape(N_OUT, D_OUT, B).transpose(2, 0, 1)  # [B, 10, 16]
    out = _squash_np(s2).astype(np.float32)
    return out, res


def kernel(x, weight):
    out, _ = run_spmd(x, weight)
    return out
